# revision 1
# baseline (speedup 1.0000x reference)
"""Trainium2 Bass kernel for causal self-attention (GQA + RoPE).

Problem: B=2, T=2048, n_embd=4096, HQ=32 q-heads, HKV=8 kv-heads, HD=128.
  q = rope(x @ wq), k = rope(x @ wk), v = x @ wv
  y = causal_softmax(q k^T / sqrt(HD)) v @ wproj

Sharding (8 cores): core = (b, g), b in {0,1} batch, g in {0..3} head-group.
Each core handles 8 q-heads / 2 kv-heads of one batch sample:
  - wq/wk/wv column-sharded, wproj row-sharded (tensor parallel over heads)
  - final reduce (sum of 4 partial y per batch) done on host in fp32.

Per-core device program (all matmuls bf16, fp32 accumulation):
  A) projections: Q^T,K^T per head [d=128 part, t free] with fused RoPE;
     V^T then PE-transposed into V[tok, dv] with a ones column appended.
  B) attention per (head, 512-token q-chunk): S^T = K-block^T-matmul(Q^T),
     causal tri mask on diag blocks, ACT exp -> P^T (bf16), then
     out[tq,129] += P^T-block.T @ [V|1] (rowsum rides in col 128),
     normalize, PE-transpose -> A^T [dv, t].
  C) y^T = wproj_s^T-blocks @ A^T, fp32 eviction, DMA out.
"""
import sys

if "/opt/trn_rl_repo" not in sys.path:
    sys.path.insert(0, "/opt/trn_rl_repo")

import math
import numpy as np
import ml_dtypes

B, T, N_EMBD = 2, 2048, 4096
HQ, HKV = 32, 8
HD = 128
N_CORES = 8
TPG = 4                      # tensor-parallel groups per batch
HQL, HKVL = HQ // TPG, HKV // TPG   # 8 q-heads, 2 kv-heads per core
SCALE = 1.0 / math.sqrt(HD)
BASE_FREQ = 10000.0
NEG = -1e30

bf16 = ml_dtypes.bfloat16


def build_nc(T=T, KE=N_EMBD, HQL=HQL, HKVL=HKVL, EOUT=N_EMBD, scale=SCALE):
    """Build the per-core Bass program. All shapes hardcoded at trace time."""
    import concourse.tile as tile
    from concourse import bacc, mybir

    f32 = mybir.dt.float32
    b16 = mybir.dt.bfloat16
    Exp = mybir.ActivationFunctionType.Exp
    mult = mybir.AluOpType.mult
    add = mybir.AluOpType.add

    KT = KE // 128          # contraction tiles for projections
    NKT = T // 128          # token tiles
    NCH = T // 512          # token chunks
    REP = HQL // HKVL

    nc = bacc.Bacc("TRN2", target_bir_lowering=False)

    xt_d = nc.dram_tensor("xt", [128, KT, T], b16, kind="ExternalInput")
    wq_d = nc.dram_tensor("wq", [128, HQL, KT, 128], b16, kind="ExternalInput")
    wk_d = nc.dram_tensor("wk", [128, HKVL, KT, 128], b16, kind="ExternalInput")
    wv_d = nc.dram_tensor("wv", [128, HKVL, KT, 128], b16, kind="ExternalInput")
    wp_d = nc.dram_tensor("wp", [128, HQL, EOUT], b16, kind="ExternalInput")
    cos_d = nc.dram_tensor("cos", [128, T], b16, kind="ExternalInput")
    sin_d = nc.dram_tensor("rsin", [64, T], b16, kind="ExternalInput")
    tri_d = nc.dram_tensor("tri", [128, 128], f32, kind="ExternalInput")
    id_d = nc.dram_tensor("ident", [128, 128], b16, kind="ExternalInput")
    yt_d = nc.dram_tensor("yt", [EOUT, T], f32, kind="ExternalOutput")

    with tile.TileContext(nc) as tc:
        with tc.tile_pool(name="glob", bufs=1) as glob:
            cos_sb = glob.tile([128, T], b16)
            sin_sb = glob.tile([64, T], b16)
            tri_sb = glob.tile([128, 128], f32)
            id_sb = glob.tile([128, 128], b16)

            qT = glob.tile([128, HQL, T], b16)       # rope(q)^T per head
            kT = glob.tile([128, HKVL, T], b16)      # rope(k)^T per head
            vON = glob.tile([128, HKVL, NKT, 129], b16)  # [tok, dv | 1]
            nc.vector.memset(vON[:, :, :, 128:129], 1.0)

            # ---------------- Phase A: projections -------------------------
            with tc.tile_pool(name="xt", bufs=1) as xtp, \
                 tc.tile_pool(name="wld", bufs=3) as wld, \
                 tc.tile_pool(name="rtmp", bufs=2) as rtmp, \
                 tc.tile_pool(name="vtmp", bufs=1) as vtmp, \
                 tc.tile_pool(name="psA", bufs=7, space="PSUM") as psA, \
                 tc.tile_pool(name="psT", bufs=1, space="PSUM") as psT:

                KH = max(KT // 2, 1)  # weight half-slab depth

                def load_w(w_d_, m):
                    wa = wld.tile([128, KH, 128], b16, tag="w", name="wa")
                    nc.sync.dma_start(out=wa[:], in_=w_d_[:, m, 0:KH, :])
                    if KT > KH:
                        wb = wld.tile([128, KH, 128], b16, tag="w", name="wb")
                        nc.sync.dma_start(out=wb[:], in_=w_d_[:, m, KH:KT, :])
                    else:
                        wb = wa
                    return lambda k: (wa[:, k, :] if k < KH
                                      else wb[:, k - KH, :])

                # First two weight slabs before xt so PE can start (and
                # keep 8 accumulators fed) while xt streams in.
                w_first = load_w(wq_d, 0)
                w_m1 = load_w(wq_d, 1)

                xt_sb = xtp.tile([128, KT, T], b16)
                for a in range(KT):
                    nc.sync.dma_start(out=xt_sb[:, a, :], in_=xt_d[:, a, :])
                    if a == KT - 1:  # tables last: ropes need them only at k=31
                        nc.sync.dma_start(out=cos_sb[:], in_=cos_d[:])
                        nc.sync.dma_start(out=sin_sb[:], in_=sin_d[:])
                        nc.sync.dma_start(out=tri_sb[:], in_=tri_d[:])
                        nc.sync.dma_start(out=id_sb[:], in_=id_d[:])

                def rope_evict(ps, dst, c):
                    # dst = ps * cos + rot64(ps) * sin  (bf16 out);
                    # rot[0:64] = -ps[64:128], rot[64:128] = ps[0:64]
                    cs = slice(512 * c, 512 * (c + 1))
                    t1 = rtmp.tile([128, 512], f32, tag="t1")
                    nc.vector.scalar_tensor_tensor(
                        t1[0:64, :], ps[64:128, :], -1.0, sin_sb[:, cs],
                        op0=mult, op1=mult)
                    nc.vector.tensor_tensor(t1[64:128, :], ps[0:64, :],
                                            sin_sb[:, cs], mult)
                    t2 = rtmp.tile([128, 512], f32, tag="t2")
                    nc.vector.tensor_tensor(t2[:], ps[:], cos_sb[:, cs], mult)
                    nc.vector.tensor_tensor(dst, t2[:], t1[:], add)

                # Startup ramp: q-heads 0+1 run k-outer, interleaved, over 7
                # live psums (m0 all 4 chunks + m1 chunks 0-2) so PE issues 7
                # matmuls per freshly-landed xt tile and tracks the DMA.
                units = [(0, c) for c in range(NCH)] + \
                        [(1, c) for c in range(NCH)]
                wfns = {0: w_first, 1: w_m1}
                pss = {u: psA.tile([128, 512], f32, tag="pj",
                                   name=f"pj{u[0]}_{u[1]}")
                       for u in units[:-1]}
                pss[units[-1]] = psT.tile([128, 512], f32, tag="tr",
                                          name="pj8")
                for k in range(KT):
                    for (m, c) in units:
                        nc.tensor.matmul(
                            pss[(m, c)][:], lhsT=wfns[m](k),
                            rhs=xt_sb[:, k, 512 * c:512 * (c + 1)],
                            start=(k == 0), stop=(k == KT - 1))
                for (m, c) in units:
                    rope_evict(pss[(m, c)], qT[:, m, 512 * c:512 * (c + 1)], c)

                # remaining projections (m1 last chunk, q-heads 2-7, k-heads)
                rest = [(qT, wq_d, m, list(range(NCH)), None)
                     for m in range(2, HQL)] + \
                    [(kT, wk_d, m, list(range(NCH)), None)
                     for m in range(HKVL)]
                for dst, w_d_, m, chunks, w_m in rest:
                    if w_m is None:
                        w_m = load_w(w_d_, m)
                    for c in chunks:
                        ps = psA.tile([128, 512], f32, tag="pj")
                        for k in range(KT):
                            nc.tensor.matmul(
                                ps[:], lhsT=w_m(k),
                                rhs=xt_sb[:, k, 512 * c:512 * (c + 1)],
                                start=(k == 0), stop=(k == KT - 1))
                        rope_evict(ps, dst[:, m, 512 * c:512 * (c + 1)], c)

                # V projection: v^T psum -> sbuf -> PE transpose -> vON
                for m in range(HKVL):
                    w_m = load_w(wv_d, m)
                    for c in range(NCH):
                        ps = psA.tile([128, 512], f32, tag="pj")
                        for k in range(KT):
                            nc.tensor.matmul(
                                ps[:], lhsT=w_m(k),
                                rhs=xt_sb[:, k, 512 * c:512 * (c + 1)],
                                start=(k == 0), stop=(k == KT - 1))
                        vt = vtmp.tile([128, 512], b16, tag="vt")
                        nc.scalar.copy(out=vt[:], in_=ps[:])
                        pt = psT.tile([128, 512], b16, tag="tr")
                        for s in range(4):
                            nc.tensor.transpose(
                                pt[:, 128 * s:128 * (s + 1)],
                                vt[:, 128 * s:128 * (s + 1)], id_sb[:])
                        for s in range(4):
                            nc.scalar.copy(
                                out=vON[:, m, 4 * c + s, 0:128],
                                in_=pt[:, 128 * s:128 * (s + 1)])

            # ---------------- Phases B + C ---------------------------------
            with tc.tile_pool(name="late", bufs=1) as late, \
                 tc.tile_pool(name="ppool", bufs=8) as ppool, \
                 tc.tile_pool(name="npool", bufs=8) as npool, \
                 tc.tile_pool(name="spool", bufs=4) as spool, \
                 tc.tile_pool(name="psS", bufs=3, space="PSUM") as psS, \
                 tc.tile_pool(name="psP", bufs=1, space="PSUM") as psP, \
                 tc.tile_pool(name="psacc", bufs=1, space="PSUM") as psacc:

                aT = late.tile([128, HQL, T], b16)
                wp_sb = late.tile([128, HQL, EOUT], b16)
                for k in range(HQL):
                    nc.sync.dma_start(out=wp_sb[:, k, :], in_=wp_d[:, k, :])

                # Phases B+C software-pipelined: while attention runs for
                # chunk c, the output projection for chunk c-1 is interleaved
                # between heads (4 e-tiles per head) so PE fills ACT-wait
                # gaps and the output DMA spreads across the whole run.
                def proj_tile(e, c, pool=None, tag="p"):
                    ps = (pool or psP).tile([128, 512], f32, tag=tag,
                                            name="psp")
                    for k in range(HQL):
                        nc.tensor.matmul(
                            ps[:], lhsT=wp_sb[:, k, 128 * e:128 * (e + 1)],
                            rhs=aT[:, k, 512 * c:512 * (c + 1)],
                            start=(k == 0), stop=(k == HQL - 1))
                    yt = ppool.tile([128, 512], f32, tag="yt", name="yt")
                    nc.vector.tensor_copy(yt[:], ps[:])
                    nc.sync.dma_start(
                        out=yt_d[128 * e:128 * (e + 1), 512 * c:512 * (c + 1)],
                        in_=yt[:])

                NE = EOUT // 128
                EPH = NE // HQL  # proj e-tiles interleaved per head
                pending = []     # deferred transpose+evict of previous head
                for c in range(NCH):
                    for h in range(HQL):
                        v = h // REP
                        # emit the previous head's A^T transposes now: their
                        # DVE normalize chain finished long ago, so PE does
                        # them back-to-back with no dependency stall.
                        for fn in pending:
                            fn()
                        pending = []
                        accs = [psacc.tile([128, 129], f32, tag=f"acc{s}",
                                           name=f"acc{s}")[:]
                                for s in range(4)]
                        n_tk = 4 * c + 4
                        pTs = {}

                        def vmms(t):
                            j = t - 4 * c
                            for s in range(4):
                                if j > s:
                                    continue
                                nc.tensor.matmul(
                                    accs[s],
                                    lhsT=pTs[t][:, 128 * s:128 * (s + 1)],
                                    rhs=vON[:, v, t, :],
                                    start=(t == 0), stop=(t == 4 * c + s))

                        # proj tiles of the previous chunk, interleaved into
                        # the t-loop (own psum bank) to fill ACT-paced gaps
                        pe_list = (list(range(EPH * h, EPH * (h + 1)))
                                   if c > 0 else [])
                        D = 4  # score->exp->V software-pipeline depth
                        for t in range(n_tk):
                            j = t - 4 * c  # >= 0 on diagonal-group tiles
                            col0 = 128 * j if j > 0 else 0
                            ps = psS.tile([128, 512], f32, tag="s")
                            nc.tensor.matmul(
                                ps[:, col0:512],
                                lhsT=kT[:, v, 128 * t:128 * (t + 1)],
                                rhs=qT[:, h, 512 * c + col0:512 * (c + 1)],
                                start=True, stop=True)
                            if j >= 0:
                                nc.vector.tensor_tensor(
                                    ps[:, 128 * j:128 * (j + 1)],
                                    ps[:, 128 * j:128 * (j + 1)],
                                    tri_sb[:], add)
                            pT = ppool.tile([128, 512], b16, tag="pT")
                            nc.scalar.activation(
                                pT[:, col0:512], ps[:, col0:512], Exp,
                                scale=scale)
                            pTs[t] = pT
                            if t >= D:
                                vmms(t - D)
                            if pe_list and \
                               (t + 1) * EPH // n_tk > t * EPH // n_tk:
                                proj_tile(pe_list.pop(0), c - 1)
                        for t in range(max(0, n_tk - D), n_tk):
                            vmms(t)
                        for e in pe_list:
                            proj_tile(e, c - 1)
                        ans = []
                        for s in range(4):
                            rec = spool.tile([128, 1], f32, tag="rec")
                            nc.vector.reciprocal(rec[:], accs[s][:, 128:129])
                            an = npool.tile([128, 128], b16, tag="an")
                            nc.vector.tensor_scalar_mul(
                                an[:], accs[s][:, 0:128], rec[:])
                            ans.append(an)

                        def make_tr(ans=ans, h=h, c=c):
                            def emit():
                                pt = psP.tile([128, 512], b16, tag="p",
                                              name="pt")
                                for s in range(4):
                                    nc.tensor.transpose(
                                        pt[:, 128 * s:128 * (s + 1)],
                                        ans[s][:], id_sb[:])
                                nc.vector.tensor_copy(
                                    aT[:, h, 512 * c:512 * (c + 1)], pt[:])
                            return emit

                        pending = [make_tr()]

                for fn in pending:
                    fn()
                # drain: projection of the last chunk. Scores are done, so
                # alternate between the proj bank and the (now idle) score
                # pool to double-buffer the drain and keep PE back-to-back.
                for e in range(NE):
                    if e % 2 == 0:
                        proj_tile(e, NCH - 1)
                    else:
                        proj_tile(e, NCH - 1, pool=psS, tag="s")

    nc.compile()
    return nc


def _rope_tables(T=T):
    j = np.arange(64, dtype=np.float64)
    inv_freq = 1.0 / (BASE_FREQ ** (2.0 * j / HD))
    t = np.arange(T, dtype=np.float64)
    fr = t[:, None] * inv_freq[None, :]          # [T, 64]
    cos = np.cos(fr)                             # cos[t, d%64]
    sin = np.sin(fr)
    cos_tbl = np.concatenate([cos, cos], axis=1).T    # [128, T]
    sin_tbl = sin.T                                   # [64, T]
    return cos_tbl.astype(bf16), sin_tbl.astype(bf16)


def _pack_w(w):
    """[KE, M] -> [128, M//128, KE//128, 128]: w_l[p, m, a, j] = w[128a+p, 128m+j]."""
    KE, M = w.shape
    return np.ascontiguousarray(
        w.reshape(KE // 128, 128, M // 128, 128).transpose(1, 2, 0, 3))


def prep_core_inputs(x, wq, wk, wv, wproj):
    cos_tbl, rsin_tbl = _rope_tables()
    tri = np.where(np.arange(128)[None, :] >= np.arange(128)[:, None],
                   0.0, NEG).astype(np.float32)
    ident = np.eye(128, dtype=bf16)
    in_maps = []
    for ci in range(N_CORES):
        b, g = divmod(ci, TPG)
        xt = np.ascontiguousarray(
            x[b].T.reshape(N_EMBD // 128, 128, T).transpose(1, 0, 2)
        ).astype(bf16)
        qcols = slice(g * HQL * HD, (g + 1) * HQL * HD)
        kvcols = slice(g * HKVL * HD, (g + 1) * HKVL * HD)
        in_maps.append({
            "xt": xt,
            "wq": _pack_w(wq[:, qcols].astype(bf16)),
            "wk": _pack_w(wk[:, kvcols].astype(bf16)),
            "wv": _pack_w(wv[:, kvcols].astype(bf16)),
            "wp": np.ascontiguousarray(
                wproj[qcols, :].reshape(HQL, 128, N_EMBD).transpose(1, 0, 2)
            ).astype(bf16),
            "cos": cos_tbl, "rsin": rsin_tbl, "tri": tri, "ident": ident,
        })
    return in_maps


_NC_CACHE = {}


def _get_nc():
    if "nc" not in _NC_CACHE:
        _NC_CACHE["nc"] = build_nc()
    return _NC_CACHE["nc"]


def _get_runner():
    """Cached sharded-jit executor over the 8 cores (no donation, so the
    compiled executable is reusable across calls)."""
    if "runner" in _NC_CACHE:
        return _NC_CACHE["runner"]
    import jax
    from jax.sharding import Mesh, PartitionSpec, NamedSharding
    from jax.experimental.shard_map import shard_map
    from concourse import mybir
    from concourse.bass2jax import (_bass_exec_p, install_neuronx_cc_hook,
                                    partition_id_tensor)

    nc = _get_nc()
    install_neuronx_cc_hook()
    pname = nc.partition_id_tensor.name if nc.partition_id_tensor else None
    in_names, out_names, out_avals, zero_shapes = [], [], [], []
    for alloc in nc.m.functions[0].allocations:
        if not isinstance(alloc, mybir.MemoryLocationSet):
            continue
        name = alloc.memorylocations[0].name
        if alloc.kind == "ExternalInput":
            if name != pname:
                in_names.append(name)
        elif alloc.kind == "ExternalOutput":
            out_names.append(name)
            shape = tuple(alloc.tensor_shape)
            dtype = mybir.dt.np(alloc.dtype)
            out_avals.append(jax.core.ShapedArray(shape, dtype))
            zero_shapes.append((shape, dtype))
    all_names = in_names + out_names + ([pname] if pname else [])

    def _body(*args):
        operands = list(args)
        if pname:
            operands.append(partition_id_tensor())
        return tuple(_bass_exec_p.bind(
            *operands, out_avals=tuple(out_avals), in_names=tuple(all_names),
            out_names=tuple(out_names), lowering_input_output_aliases=(),
            sim_require_finite=True, sim_require_nnan=True, nc=nc))

    devices = jax.devices()[:N_CORES]
    mesh = Mesh(np.asarray(devices), ("core",))
    nin = len(in_names) + len(out_names)
    sharded = jax.jit(
        shard_map(_body, mesh=mesh, in_specs=(PartitionSpec("core"),) * nin,
                  out_specs=(PartitionSpec("core"),) * len(out_names),
                  check_rep=False),
        keep_unused=True)
    sh = NamedSharding(mesh, PartitionSpec("core"))
    zeros = [jax.device_put(
        np.zeros((N_CORES * s[0], *s[1:]), dt), sh)
        for s, dt in zero_shapes]

    def run(in_maps):
        concat = [np.concatenate([m[n] for m in in_maps], axis=0)
                  for n in in_names]
        dev_in = [jax.device_put(a, sh) for a in concat]
        outs = sharded(*dev_in, *zeros)
        jax.block_until_ready(outs)
        return [
            {n: np.asarray(outs[i]).reshape(N_CORES, *out_avals[i].shape)[ci]
             for i, n in enumerate(out_names)}
            for ci in range(N_CORES)]

    _NC_CACHE["runner"] = run
    return run


def kernel(x, wq, wk, wv, wproj):
    in_maps = prep_core_inputs(np.asarray(x, dtype=np.float32),
                               np.asarray(wq, dtype=np.float32),
                               np.asarray(wk, dtype=np.float32),
                               np.asarray(wv, dtype=np.float32),
                               np.asarray(wproj, dtype=np.float32))
    results = _get_runner()(in_maps)
    y = np.empty((B, T, N_EMBD), dtype=np.float32)
    for b in range(B):
        acc = results[b * TPG]["yt"].copy()
        for g in range(1, TPG):
            acc += results[b * TPG + g]["yt"]
        y[b] = acc.T
    return y


if __name__ == "__main__":
    rng = np.random.default_rng(0)
    x = rng.standard_normal((B, T, N_EMBD), dtype=np.float32)
    wq_ = (rng.standard_normal((N_EMBD, N_EMBD), dtype=np.float32) * 0.02)
    wk_ = (rng.standard_normal((N_EMBD, HKV * HD), dtype=np.float32) * 0.02)
    wv_ = (rng.standard_normal((N_EMBD, HKV * HD), dtype=np.float32) * 0.02)
    wp_ = (rng.standard_normal((N_EMBD, N_EMBD), dtype=np.float32) * 0.02)
    y = kernel(x, wq_, wk_, wv_, wp_)
    print("out", y.shape, y.dtype, float(np.abs(y).max()))



# revision 55
# speedup vs baseline: 1.1678x; 1.1678x over previous
"""Trainium2 Bass kernel for causal self-attention (GQA + RoPE).

Problem: B=2, T=2048, n_embd=4096, HQ=32 q-heads, HKV=8 kv-heads, HD=128.
  q = rope(x @ wq), k = rope(x @ wk), v = x @ wv
  y = causal_softmax(q k^T / sqrt(HD)) v @ wproj

Sharding (8 cores): core = (b, g), b in {0,1} batch, g in {0..3} head-group.
Each core handles 8 q-heads / 2 kv-heads of one batch sample:
  - wq/wk/wv column-sharded, wproj row-sharded (tensor parallel over heads)
  - final reduce (sum of 4 partial y per batch) done on host in fp32.

Per-core device program:
  A) projections in COMPENSATED fp8 (e4m3 hi+lo splits of x and w, x64
     weight scaling folded into the rope tables / V eviction): per
     (head, chunk) one PSUM accumulates 48 DoubleRow matmuls
     (3 products x 16 k-tile pairs, 256-deep contraction each), then
     fused RoPE evict (bf16 out).  V^T is PE-transposed into V[tok, dv]
     with a ones column appended.
  B) attention per (head, 512-token q-chunk) in bf16: S^T =
     K-block^T-matmul(Q^T), causal tri mask on diag blocks, ACT exp ->
     P^T (bf16), then out[tq,129] += P^T-block.T @ [V|1] (rowsum rides
     in col 128), normalize, split into fp8 hi+lo, PE-transpose ->
     A^T_hi/A^T_lo [dv, t].
  C) y^T = compensated fp8 DoubleRow over head pairs:
     3 products x 4 head-pairs per e-tile, fp32 eviction, DMA out.
"""
import sys

if "/opt/trn_rl_repo" not in sys.path:
    sys.path.insert(0, "/opt/trn_rl_repo")

import math
import numpy as np
import ml_dtypes

B, T, N_EMBD = 2, 2048, 4096
HQ, HKV = 32, 8
HD = 128
N_CORES = 8
TPG = 4                      # tensor-parallel groups per batch
HQL, HKVL = HQ // TPG, HKV // TPG   # 8 q-heads, 2 kv-heads per core
SCALE = 1.0 / math.sqrt(HD)
BASE_FREQ = 10000.0
NEG = -1e30
SW = 64.0                    # fp8 weight pre-scale

bf16 = ml_dtypes.bfloat16
f8e4 = ml_dtypes.float8_e4m3


def build_nc(T=T, KE=N_EMBD, HQL=HQL, HKVL=HKVL, EOUT=N_EMBD, scale=SCALE):
    """Build the per-core Bass program. All shapes hardcoded at trace time."""
    import concourse.tile as tile
    from concourse import bacc, mybir

    f32 = mybir.dt.float32
    b16 = mybir.dt.bfloat16
    fp8 = mybir.dt.float8e4
    Exp = mybir.ActivationFunctionType.Exp
    Copy = mybir.ActivationFunctionType.Copy
    DR = mybir.MatmulPerfMode.DoubleRow
    mult = mybir.AluOpType.mult
    add = mybir.AluOpType.add
    sub = mybir.AluOpType.subtract

    KT = KE // 128          # contraction tiles for projections
    KP = KT // 2            # DoubleRow k-tile pairs
    NKT = T // 128          # token tiles
    NCH = T // 512          # token chunks
    REP = HQL // HKVL

    nc = bacc.Bacc("TRN2", target_bir_lowering=False)

    xh_d = nc.dram_tensor("xh", [128, KT, T], fp8, kind="ExternalInput")
    xl_d = nc.dram_tensor("xl", [128, KT, T], fp8, kind="ExternalInput")
    wqh_d = nc.dram_tensor("wqh", [128, HQL, KT, 128], fp8, kind="ExternalInput")
    wql_d = nc.dram_tensor("wql", [128, HQL, KT, 128], fp8, kind="ExternalInput")
    wkh_d = nc.dram_tensor("wkh", [128, HKVL, KT, 128], fp8, kind="ExternalInput")
    wkl_d = nc.dram_tensor("wkl", [128, HKVL, KT, 128], fp8, kind="ExternalInput")
    wvh_d = nc.dram_tensor("wvh", [128, HKVL, KT, 128], fp8, kind="ExternalInput")
    wvl_d = nc.dram_tensor("wvl", [128, HKVL, KT, 128], fp8, kind="ExternalInput")
    NE = EOUT // 128
    wph_d = nc.dram_tensor("wph", [128, NE, HQL, 128], fp8, kind="ExternalInput")
    wpl_d = nc.dram_tensor("wpl", [128, NE, HQL, 128], fp8, kind="ExternalInput")
    cos_d = nc.dram_tensor("cos", [128, T], b16, kind="ExternalInput")
    sin_d = nc.dram_tensor("rsin", [64, T], b16, kind="ExternalInput")
    tri_d = nc.dram_tensor("tri", [128, 128], f32, kind="ExternalInput")
    id_d = nc.dram_tensor("ident", [128, 128], b16, kind="ExternalInput")
    yt_d = nc.dram_tensor("yt", [EOUT, T], f32, kind="ExternalOutput")

    with tile.TileContext(nc) as tc:
        with tc.tile_pool(name="glob", bufs=1) as glob:
            cos_sb = glob.tile([128, T], b16)
            sin_sb = glob.tile([64, T], b16)
            tri_sb = glob.tile([128, 128], f32)
            id_sb = glob.tile([128, 128], b16)

            qT = glob.tile([128, HQL, T], b16)       # rope(q)^T per head
            kT = glob.tile([128, HKVL, T], b16)      # rope(k)^T per head
            vON = glob.tile([128, HKVL, NKT, 129], b16)  # [tok, dv | 1]
            nc.vector.memset(vON[:, :, :, 128:129], 1.0)

            # ---------------- Phase A: projections -------------------------
            with tc.tile_pool(name="xt", bufs=1) as xtp, \
                 tc.tile_pool(name="wld", bufs=8) as wld, \
                 tc.tile_pool(name="rtmp", bufs=1) as rtmp, \
                 tc.tile_pool(name="vtmp", bufs=1) as vtmp, \
                 tc.tile_pool(name="psA", bufs=7, space="PSUM") as psA, \
                 tc.tile_pool(name="psT", bufs=1, space="PSUM") as psT:

                KH = KT // 2  # weight half-slab depth (16 tiles)

                def load_w(wh_d_, wl_d_, m):
                    """Returns f(prod, kp) -> [128, 2, 128] AP of the k-pair.
                    prod 0 -> hi weights, 1 -> lo weights."""
                    slabs = {}
                    for key, w_d_ in (("h", wh_d_), ("l", wl_d_)):
                        wa = wld.tile([128, KH, 128], fp8, tag="w",
                                      name=f"wa{key}")
                        nc.sync.dma_start(out=wa[:], in_=w_d_[:, m, 0:KH, :])
                        wb = wld.tile([128, KH, 128], fp8, tag="w",
                                      name=f"wb{key}")
                        nc.sync.dma_start(out=wb[:], in_=w_d_[:, m, KH:KT, :])
                        slabs[key] = (wa, wb)

                    def get(key, kp):
                        wa, wb = slabs[key]
                        k0 = 2 * kp
                        if k0 < KH:
                            return wa[:, k0:k0 + 2, :]
                        return wb[:, k0 - KH:k0 - KH + 2, :]
                    return get

                # DMA issue order tracks first use: the m0/m1 hi a-slabs and
                # first x tiles land first (first matmul ~2.5us in); lo
                # a-slabs early because passes 1+2 run interleaved per
                # k-pair (pass 2 reuses the same xh tiles); big rope tables
                # follow the b-slabs (first rope is ~40us in); xl last.
                xh_sb = xtp.tile([128, KT, T], fp8)
                xl_sb = xtp.tile([128, KT, T], fp8)

                def load_slab(w_d_, m, half):
                    w = wld.tile([128, KH, 128], fp8, tag="w")
                    lo, hi = (0, KH) if half == 0 else (KH, KT)
                    nc.sync.dma_start(out=w[:], in_=w_d_[:, m, lo:hi, :])
                    return w

                wa0h = load_slab(wqh_d, 0, 0)
                nc.sync.dma_start(out=xh_sb[:, 0, :], in_=xh_d[:, 0, :])
                nc.sync.dma_start(out=xh_sb[:, 1, :], in_=xh_d[:, 1, :])
                wa1h = load_slab(wqh_d, 1, 0)
                wa0l = load_slab(wql_d, 0, 0)
                wa1l = load_slab(wql_d, 1, 0)
                for a in range(2, KT):
                    nc.sync.dma_start(out=xh_sb[:, a, :], in_=xh_d[:, a, :])
                    if a == 10:  # b-halves needed from kp=8
                        wb0h = load_slab(wqh_d, 0, 1)
                        wb1h = load_slab(wqh_d, 1, 1)
                        wb0l = load_slab(wql_d, 0, 1)
                        wb1l = load_slab(wql_d, 1, 1)
                nc.sync.dma_start(out=cos_sb[:], in_=cos_d[:])
                nc.sync.dma_start(out=sin_sb[:], in_=sin_d[:])
                nc.sync.dma_start(out=tri_sb[:], in_=tri_d[:])
                nc.sync.dma_start(out=id_sb[:], in_=id_d[:])
                for a in range(KT):
                    nc.sync.dma_start(out=xl_sb[:, a, :], in_=xl_d[:, a, :])

                def mk_wfn(wa, wb, wal, wbl):
                    slabs = {"h": (wa, wb), "l": (wal, wbl)}

                    def get(key, kp):
                        a, b = slabs[key]
                        k0 = 2 * kp
                        if k0 < KH:
                            return a[:, k0:k0 + 2, :]
                        return b[:, k0 - KH:k0 - KH + 2, :]
                    return get

                def rope_evict(ps, dst, c):
                    # dst = ps * cos + rot64(ps) * sin  (bf16 out);
                    # rot[0:64] = -ps[64:128], rot[64:128] = ps[0:64]
                    cs = slice(512 * c, 512 * (c + 1))
                    t1 = rtmp.tile([128, 512], f32, tag="t1")
                    nc.vector.scalar_tensor_tensor(
                        t1[0:64, :], ps[64:128, :], -1.0, sin_sb[:, cs],
                        op0=mult, op1=mult)
                    nc.vector.tensor_tensor(t1[64:128, :], ps[0:64, :],
                                            sin_sb[:, cs], mult)
                    t2 = rtmp.tile([128, 512], f32, tag="t2")
                    nc.vector.tensor_tensor(t2[:], ps[:], cos_sb[:, cs], mult)
                    nc.vector.tensor_tensor(dst, t2[:], t1[:], add)

                # (prod, x-operand) sequence for the compensated product:
                #   x_hi@w_hi + x_hi@w_lo + x_lo@w_hi
                def dr_chain(ps, wfn, c, kp_order=None):
                    cs = slice(512 * c, 512 * (c + 1))
                    steps = [("h", xh_sb), ("l", xh_sb), ("h", xl_sb)]
                    n = 0
                    for si, (wkey, xsb) in enumerate(steps):
                        for kp in range(KP):
                            nc.tensor.matmul(
                                ps[:], lhsT=wfn(wkey, kp),
                                rhs=xsb[:, 2 * kp:2 * kp + 2, cs],
                                start=(n == 0), stop=(n == 3 * KP - 1),
                                perf_mode=DR)
                            n += 1

                # Startup ramp: q-heads 0+1 run kp-outer, interleaved, over 8
                # live psums so PE issues 8 matmuls per freshly-landed x tile
                # and tracks the DMA (pass 1 follows xh, pass 3 follows xl).
                # The last pass runs unit-major so early units stop (and
                # their rope evicts start on DVE) while PE finishes the rest.
                units = [(0, c) for c in range(NCH)] + \
                        [(1, c) for c in range(NCH)]
                wfns = {0: mk_wfn(wa0h, wb0h, wa0l, wb0l),
                        1: mk_wfn(wa1h, wb1h, wa1l, wb1l)}
                pss = {u: psA.tile([128, 512], f32, tag="pj",
                                   name=f"pj{u[0]}_{u[1]}")
                       for u in units[:-1]}
                pss[units[-1]] = psT.tile([128, 512], f32, tag="tr",
                                          name="pj8")
                # passes 1+2 interleaved per k-pair: both read the same two
                # xh tiles, so PE does 16 matmuls per 2-tile DMA landing and
                # stays ahead of the x stream.
                for kp in range(KP):
                    for wkey in ("h", "l"):
                        for (m, c) in units:
                            nc.tensor.matmul(
                                pss[(m, c)][:], lhsT=wfns[m](wkey, kp),
                                rhs=xh_sb[:, 2 * kp:2 * kp + 2,
                                          512 * c:512 * (c + 1)],
                                start=(wkey == "h" and kp == 0), stop=False,
                                perf_mode=DR)
                for (m, c) in units:
                    for kp in range(KP):
                        nc.tensor.matmul(
                            pss[(m, c)][:], lhsT=wfns[m]("h", kp),
                            rhs=xl_sb[:, 2 * kp:2 * kp + 2,
                                      512 * c:512 * (c + 1)],
                            start=False, stop=(kp == KP - 1),
                            perf_mode=DR)
                    rope_evict(pss[(m, c)], qT[:, m, 512 * c:512 * (c + 1)], c)

                # remaining projections: q-heads 2-7, k-heads, then V heads.
                # Next head's weight slabs are prefetched before the current
                # head's chunk chains are issued (ring bufs sized to hold
                # current 4 + next 4 slabs).
                heads = [("r", qT, wqh_d, wql_d, m) for m in range(2, HQL)] + \
                        [("r", kT, wkh_d, wkl_d, m) for m in range(HKVL)] + \
                        [("v", None, wvh_d, wvl_d, m) for m in range(HKVL)]
                wcur = load_w(heads[0][2], heads[0][3], heads[0][4])
                for i, (kind, dst, wh_d_, wl_d_, m) in enumerate(heads):
                    wnxt = (load_w(heads[i + 1][2], heads[i + 1][3],
                                   heads[i + 1][4])
                            if i + 1 < len(heads) else None)
                    for c in range(NCH):
                        ps = psA.tile([128, 512], f32, tag="pj")
                        dr_chain(ps, wcur, c)
                        if kind == "r":
                            rope_evict(ps, dst[:, m, 512 * c:512 * (c + 1)],
                                       c)
                        else:
                            # V: v^T psum -> (1/SW) sbuf -> PE transpose
                            vt = vtmp.tile([128, 512], b16, tag="vt")
                            nc.scalar.activation(vt[:], ps[:], Copy,
                                                 scale=1.0 / SW)
                            pt = psT.tile([128, 512], b16, tag="tr")
                            for s in range(4):
                                nc.tensor.transpose(
                                    pt[:, 128 * s:128 * (s + 1)],
                                    vt[:, 128 * s:128 * (s + 1)], id_sb[:])
                            for s in range(4):
                                nc.scalar.copy(
                                    out=vON[:, m, 4 * c + s, 0:128],
                                    in_=pt[:, 128 * s:128 * (s + 1)])
                    wcur = wnxt

            # ---------------- Phases B + C ---------------------------------
            with tc.tile_pool(name="late", bufs=1) as late, \
                 tc.tile_pool(name="ppool", bufs=10) as ppool, \
                 tc.tile_pool(name="npool", bufs=8) as npool, \
                 tc.tile_pool(name="spool", bufs=4) as spool, \
                 tc.tile_pool(name="psS", bufs=3, space="PSUM") as psS, \
                 tc.tile_pool(name="psP", bufs=1, space="PSUM") as psP, \
                 tc.tile_pool(name="psacc", bufs=1, space="PSUM") as psacc:

                aTh = late.tile([128, HQL, T], fp8)
                aTl = late.tile([128, HQL, T], fp8)
                # wproj packed per e-column-tile so DMA lands in consumption
                # order (proj tiles only need their own e slabs, not all of
                # wproj, when the phase-A -> B transition is DMA-tight)
                wph_sb = late.tile([128, NE, HQL, 128], fp8)
                wpl_sb = late.tile([128, NE, HQL, 128], fp8)
                for e in range(NE):
                    nc.sync.dma_start(out=wph_sb[:, e], in_=wph_d[:, e])
                    nc.sync.dma_start(out=wpl_sb[:, e], in_=wpl_d[:, e])

                # Phases B+C software-pipelined: while attention runs for
                # chunk c, the output projection for chunk c-1 is interleaved
                # between heads (4 e-tiles per head) so PE fills ACT-wait
                # gaps and the output DMA spreads across the whole run.
                def proj_tile(e, c, drain=False):
                    cs = slice(512 * c, 512 * (c + 1))
                    ps = psS.tile([128, 512], f32, tag="s", name="psp")
                    n = 0
                    for wsb, asb in ((wph_sb, aTh), (wpl_sb, aTh),
                                     (wph_sb, aTl)):
                        for hp in range(HQL // 2):
                            nc.tensor.matmul(
                                ps[:], lhsT=wsb[:, e, 2 * hp:2 * hp + 2, :],
                                rhs=asb[:, 2 * hp:2 * hp + 2, cs],
                                start=(n == 0), stop=(n == 3 * HQL // 2 - 1),
                                perf_mode=DR)
                            n += 1
                    yt = ppool.tile([128, 512], f32, tag="yt", name="yt")
                    # in-loop evicts stay off ACT (it paces the exp chain);
                    # the drain has no exps so it alternates
                    if drain and e % 2 == 1:
                        nc.scalar.copy(out=yt[:], in_=ps[:])
                    else:
                        nc.vector.tensor_copy(yt[:], ps[:])
                    nc.sync.dma_start(
                        out=yt_d[128 * e:128 * (e + 1), 512 * c:512 * (c + 1)],
                        in_=yt[:])

                NE = EOUT // 128
                EPH = NE // HQL  # proj e-tiles interleaved per head
                # carry: [(emit_ti, fn)] — work of the PREVIOUS head (tail
                # AV matmuls + normalize, then A^T transposes) deferred into
                # the current head's t-loop so PE never waits on it inline.
                carry = []
                for c in range(NCH):
                    for h in range(HQL):
                        v = h // REP
                        accs = [psacc.tile([128, 129], f32, tag=f"acc{s}",
                                           name=f"acc{s}")[:]
                                for s in range(4)]
                        n_tk = 4 * c + 4
                        pTs = {}

                        def vmms(t, accs=accs, pTs=pTs, v=v, c=c):
                            j = t - 4 * c
                            for s in range(4):
                                if j > s:
                                    continue
                                # emission order: fulls interleaved with
                                # diags, ending [..., full 4c-1, diag 4c+3]
                                stop_t = (s if c == 0 else
                                          (4 * c + 3 if s == 3 else
                                           4 * c - 1))
                                nc.tensor.matmul(
                                    accs[s],
                                    lhsT=pTs[t][:, 128 * s:128 * (s + 1)],
                                    rhs=vON[:, v, t, :],
                                    start=(t == 0), stop=(t == stop_t))

                        # proj tiles of the previous chunk, interleaved into
                        # the t-loop (own psum bank) to fill ACT-paced gaps
                        pe_list = (list(range(EPH * h, EPH * (h + 1)))
                                   if c > 0 else [])
                        D = 4 if c > 0 else 3
                        # diag tiles spread through the loop (one after
                        # every c full tiles): their ACT exp work and DVE
                        # tri-adds never bunch at the tail, and the loop
                        # ends on the smallest (128-col) exp.
                        t_seq = []
                        for i in range(4):
                            t_seq += list(range(c * i, c * (i + 1)))
                            t_seq.append(4 * c + i)
                        for ti, t in enumerate(t_seq):
                            j = t - 4 * c  # >= 0 on diagonal-group tiles
                            col0 = 128 * j if j > 0 else 0
                            ps = psS.tile([128, 512], f32, tag="s")
                            nc.tensor.matmul(
                                ps[:, col0:512],
                                lhsT=kT[:, v, 128 * t:128 * (t + 1)],
                                rhs=qT[:, h, 512 * c + col0:512 * (c + 1)],
                                start=True, stop=True)
                            while carry and carry[0][0] <= ti:
                                carry.pop(0)[1]()
                            if j >= 0:
                                nc.vector.tensor_tensor(
                                    ps[:, 128 * j:128 * (j + 1)],
                                    ps[:, 128 * j:128 * (j + 1)],
                                    tri_sb[:], add)
                            pT = ppool.tile([128, 512], b16, tag="pT")
                            nc.scalar.activation(
                                pT[:, col0:512], ps[:, col0:512], Exp,
                                scale=scale)
                            pTs[t] = pT
                            if ti >= D:
                                vmms(t_seq[ti - D])
                            # proj tiles read ALL heads' aT of chunk c-1;
                            # at h==0 the previous chunk's last head's
                            # transposes are only emitted at ti==6, so its
                            # proj tiles must come after (reads emitted
                            # before writes get no dependency edge)
                            if pe_list and (h > 0 or ti >= 8) and \
                               (ti + 1) * EPH // (n_tk + 3) > \
                               ti * EPH // (n_tk + 3):
                                proj_tile(pe_list.pop(0), c - 1)
                        for e in pe_list:
                            proj_tile(e, c - 1)

                        # Package this head's tail: the last D AV matmuls
                        # (their exps are still draining on ACT) plus the
                        # normalize chain; and, later, the A^T transposes.
                        # Both run inside the NEXT head's t-loop.
                        holder = []

                        def make_tail(vmms=vmms, accs=accs, n_tk=n_tk, D=D,
                                      holder=holder, t_seq=tuple(t_seq)):
                            def emit():
                                for t in t_seq[max(0, n_tk - D):]:
                                    vmms(t)
                                # batched normalize: 4 s-blocks land in one
                                # [128,512] bf16 tile; the fp8 hi/lo split
                                # happens AFTER the transpose (identical
                                # math, and bf16 PE transposes are legal
                                # where fp8 ones need stride-2 outputs)
                                an = npool.tile([128, 512], b16, tag="an")
                                for s in range(4):
                                    rec = spool.tile([128, 1], f32,
                                                     tag="rec")
                                    nc.vector.reciprocal(
                                        rec[:], accs[s][:, 128:129])
                                    nc.vector.tensor_scalar_mul(
                                        an[:, 128 * s:128 * (s + 1)],
                                        accs[s][:, 0:128], rec[:])
                                holder.append(an)
                            return emit

                        def make_tr(holder=holder, h=h, c=c):
                            def emit():
                                pt = psP.tile([128, 512], b16, tag="p8",
                                              name="pt8")
                                an = holder[0]
                                for s in range(4):
                                    nc.tensor.transpose(
                                        pt[:, 128 * s:128 * (s + 1)],
                                        an[:, 128 * s:128 * (s + 1)],
                                        id_sb[:])
                                cs = slice(512 * c, 512 * (c + 1))
                                # post-transpose hi/lo split: hi on ACT,
                                # lo = pt - hi on DVE
                                nc.scalar.copy(out=aTh[:, h, cs],
                                               in_=pt[:])
                                nc.vector.tensor_tensor(
                                    aTl[:, h, cs], pt[:], aTh[:, h, cs],
                                    sub)
                            return emit

                        for _, fn in carry:  # flush any unemitted leftovers
                            fn()
                        # emit points must fit inside the NEXT iteration's
                        # t-loop (n_tk=4 when it is a c==0 head)
                        nxt_c0 = (c == 0 and h < HQL - 1)
                        carry = [(1, make_tail()),
                                 (3 if nxt_c0 else 6, make_tr())]

                for _, fn in carry:
                    fn()
                # drain: projection of the last chunk through the 3-bank ring
                for e in range(NE):
                    proj_tile(e, NCH - 1)

    nc.compile()
    return nc


def _rope_tables(T=T):
    j = np.arange(64, dtype=np.float64)
    inv_freq = 1.0 / (BASE_FREQ ** (2.0 * j / HD))
    t = np.arange(T, dtype=np.float64)
    fr = t[:, None] * inv_freq[None, :]          # [T, 64]
    cos = np.cos(fr) / SW                        # fold 1/SW (fp8 w scaling)
    sin = np.sin(fr) / SW
    cos_tbl = np.concatenate([cos, cos], axis=1).T    # [128, T]
    sin_tbl = sin.T                                   # [64, T]
    return cos_tbl.astype(bf16), sin_tbl.astype(bf16)


def _hilo(a):
    """fp8 e4m3 hi/lo split of a float32 array."""
    h = a.astype(f8e4)
    l = (a - h.astype(np.float32)).astype(f8e4)
    return h, l


def _pack_w(w):
    """[KE, M] -> [128, M//128, KE//128, 128]: w_l[p, m, a, j] = w[128a+p, 128m+j]."""
    KE, M = w.shape
    return np.ascontiguousarray(
        w.reshape(KE // 128, 128, M // 128, 128).transpose(1, 2, 0, 3))


def prep_core_inputs(x, wq, wk, wv, wproj):
    cos_tbl, rsin_tbl = _rope_tables()
    tri = np.where(np.arange(128)[None, :] >= np.arange(128)[:, None],
                   0.0, NEG).astype(np.float32)
    ident = np.eye(128, dtype=bf16)

    # shared fp8 splits (sliced per core below)
    wqh, wql = _hilo(wq * SW)
    wkh, wkl = _hilo(wk * SW)
    wvh, wvl = _hilo(wv * SW)
    wph, wpl = _hilo(wproj * SW)

    xs = []
    for b in range(B):
        xt = np.ascontiguousarray(
            x[b].T.reshape(N_EMBD // 128, 128, T).transpose(1, 0, 2))
        xs.append(_hilo(xt.astype(np.float32)))

    def packp(w, cols):
        # [1024, E] -> [128, NE, HQL, 128]: per e-column-tile slabs
        return np.ascontiguousarray(
            w[cols, :].reshape(HQL, 128, N_EMBD // 128, 128)
            .transpose(1, 2, 0, 3))

    in_maps = []
    for ci in range(N_CORES):
        b, g = divmod(ci, TPG)
        qcols = slice(g * HQL * HD, (g + 1) * HQL * HD)
        kvcols = slice(g * HKVL * HD, (g + 1) * HKVL * HD)
        in_maps.append({
            "xh": xs[b][0], "xl": xs[b][1],
            "wqh": _pack_w(wqh[:, qcols]), "wql": _pack_w(wql[:, qcols]),
            "wkh": _pack_w(wkh[:, kvcols]), "wkl": _pack_w(wkl[:, kvcols]),
            "wvh": _pack_w(wvh[:, kvcols]), "wvl": _pack_w(wvl[:, kvcols]),
            "wph": packp(wph, qcols), "wpl": packp(wpl, qcols),
            "cos": cos_tbl, "rsin": rsin_tbl, "tri": tri, "ident": ident,
        })
    return in_maps


_NC_CACHE = {}


def _get_nc():
    if "nc" not in _NC_CACHE:
        _NC_CACHE["nc"] = build_nc()
    return _NC_CACHE["nc"]


def _get_runner():
    """Cached sharded-jit executor over the 8 cores (no donation, so the
    compiled executable is reusable across calls)."""
    if "runner" in _NC_CACHE:
        return _NC_CACHE["runner"]
    import jax
    from jax.sharding import Mesh, PartitionSpec, NamedSharding
    from jax.experimental.shard_map import shard_map
    from concourse import mybir
    from concourse.bass2jax import (_bass_exec_p, install_neuronx_cc_hook,
                                    partition_id_tensor)

    nc = _get_nc()
    install_neuronx_cc_hook()
    pname = nc.partition_id_tensor.name if nc.partition_id_tensor else None
    in_names, out_names, out_avals, zero_shapes = [], [], [], []
    for alloc in nc.m.functions[0].allocations:
        if not isinstance(alloc, mybir.MemoryLocationSet):
            continue
        name = alloc.memorylocations[0].name
        if alloc.kind == "ExternalInput":
            if name != pname:
                in_names.append(name)
        elif alloc.kind == "ExternalOutput":
            out_names.append(name)
            shape = tuple(alloc.tensor_shape)
            dtype = mybir.dt.np(alloc.dtype)
            out_avals.append(jax.core.ShapedArray(shape, dtype))
            zero_shapes.append((shape, dtype))
    all_names = in_names + out_names + ([pname] if pname else [])

    def _body(*args):
        operands = list(args)
        if pname:
            operands.append(partition_id_tensor())
        return tuple(_bass_exec_p.bind(
            *operands, out_avals=tuple(out_avals), in_names=tuple(all_names),
            out_names=tuple(out_names), lowering_input_output_aliases=(),
            sim_require_finite=True, sim_require_nnan=True, nc=nc))

    devices = jax.devices()[:N_CORES]
    mesh = Mesh(np.asarray(devices), ("core",))
    nin = len(in_names) + len(out_names)
    sharded = jax.jit(
        shard_map(_body, mesh=mesh, in_specs=(PartitionSpec("core"),) * nin,
                  out_specs=(PartitionSpec("core"),) * len(out_names),
                  check_rep=False),
        keep_unused=True)
    sh = NamedSharding(mesh, PartitionSpec("core"))
    zeros = [jax.device_put(
        np.zeros((N_CORES * s[0], *s[1:]), dt), sh)
        for s, dt in zero_shapes]

    def run(in_maps):
        concat = [np.concatenate([m[n] for m in in_maps], axis=0)
                  for n in in_names]
        dev_in = [jax.device_put(a, sh) for a in concat]
        outs = sharded(*dev_in, *zeros)
        jax.block_until_ready(outs)
        return [
            {n: np.asarray(outs[i]).reshape(N_CORES, *out_avals[i].shape)[ci]
             for i, n in enumerate(out_names)}
            for ci in range(N_CORES)]

    _NC_CACHE["runner"] = run
    return run


def kernel(x, wq, wk, wv, wproj):
    in_maps = prep_core_inputs(np.asarray(x, dtype=np.float32),
                               np.asarray(wq, dtype=np.float32),
                               np.asarray(wk, dtype=np.float32),
                               np.asarray(wv, dtype=np.float32),
                               np.asarray(wproj, dtype=np.float32))
    results = _get_runner()(in_maps)
    y = np.empty((B, T, N_EMBD), dtype=np.float32)
    for b in range(B):
        acc = results[b * TPG]["yt"].copy()
        for g in range(1, TPG):
            acc += results[b * TPG + g]["yt"]
        y[b] = acc.T / SW
    return y


if __name__ == "__main__":
    rng = np.random.default_rng(0)
    x = rng.standard_normal((B, T, N_EMBD), dtype=np.float32)
    wq_ = (rng.standard_normal((N_EMBD, N_EMBD), dtype=np.float32) * 0.02)
    wk_ = (rng.standard_normal((N_EMBD, HKV * HD), dtype=np.float32) * 0.02)
    wv_ = (rng.standard_normal((N_EMBD, HKV * HD), dtype=np.float32) * 0.02)
    wp_ = (rng.standard_normal((N_EMBD, N_EMBD), dtype=np.float32) * 0.02)
    y = kernel(x, wq_, wk_, wv_, wp_)
    print("out", y.shape, y.dtype, float(np.abs(y).max()))


# revision 60
# speedup vs baseline: 1.2484x; 1.0689x over previous
"""Trainium2 Bass kernel for causal self-attention (GQA + RoPE).

Problem: B=2, T=2048, n_embd=4096, HQ=32 q-heads, HKV=8 kv-heads, HD=128.
  q = rope(x @ wq), k = rope(x @ wk), v = x @ wv
  y = causal_softmax(q k^T / sqrt(HD)) v @ wproj

Sharding (8 cores): core = (b, g), b in {0,1} batch, g in {0..3} head-group.
Each core handles 8 q-heads / 2 kv-heads of one batch sample:
  - wq/wk/wv column-sharded, wproj row-sharded (tensor parallel over heads)
  - final reduce (sum of 4 partial y per batch) done on host in fp32.

Per-core device program:
  A) projections in COMPENSATED fp8 (e4m3 hi+lo splits of x and w, x64
     weight scaling folded into the rope tables / V eviction): per
     (head, chunk) one PSUM accumulates 48 DoubleRow matmuls
     (3 products x 16 k-tile pairs, 256-deep contraction each), then
     fused RoPE evict (bf16 out).  V^T is PE-transposed into V[tok, dv]
     with a ones column appended.
  B) attention per (head, 512-token q-chunk) in bf16: S^T =
     K-block^T-matmul(Q^T), causal tri mask on diag blocks, ACT exp ->
     P^T (bf16), then out[tq,129] += P^T-block.T @ [V|1] (rowsum rides
     in col 128), normalize, split into fp8 hi+lo, PE-transpose ->
     A^T_hi/A^T_lo [dv, t].
  C) y^T = compensated fp8 DoubleRow over head pairs:
     3 products x 4 head-pairs per e-tile, fp32 eviction, DMA out.
"""
import sys

if "/opt/trn_rl_repo" not in sys.path:
    sys.path.insert(0, "/opt/trn_rl_repo")

import math
import numpy as np
import ml_dtypes

B, T, N_EMBD = 2, 2048, 4096
HQ, HKV = 32, 8
HD = 128
N_CORES = 8
TPG = 4                      # tensor-parallel groups per batch
HQL, HKVL = HQ // TPG, HKV // TPG   # 8 q-heads, 2 kv-heads per core
SCALE = 1.0 / math.sqrt(HD)
BASE_FREQ = 10000.0
NEG = -1e30
SW = 64.0                    # fp8 weight pre-scale

bf16 = ml_dtypes.bfloat16
f8e4 = ml_dtypes.float8_e4m3


def build_nc(T=T, KE=N_EMBD, HQL=HQL, HKVL=HKVL, EOUT=N_EMBD, scale=SCALE):
    """Build the per-core Bass program. All shapes hardcoded at trace time."""
    import concourse.tile as tile
    from concourse import bacc, mybir

    f32 = mybir.dt.float32
    b16 = mybir.dt.bfloat16
    fp8 = mybir.dt.float8e4
    Exp = mybir.ActivationFunctionType.Exp
    Copy = mybir.ActivationFunctionType.Copy
    DR = mybir.MatmulPerfMode.DoubleRow
    mult = mybir.AluOpType.mult
    add = mybir.AluOpType.add
    sub = mybir.AluOpType.subtract

    KT = KE // 128          # contraction tiles for projections
    KP = KT // 2            # DoubleRow k-tile pairs
    NKT = T // 128          # token tiles
    NCH = T // 512          # token chunks
    REP = HQL // HKVL

    nc = bacc.Bacc("TRN2", target_bir_lowering=False)

    xh_d = nc.dram_tensor("xh", [128, KT, T], fp8, kind="ExternalInput")
    xl_d = nc.dram_tensor("xl", [128, KT, T], fp8, kind="ExternalInput")
    wqh_d = nc.dram_tensor("wqh", [128, HQL, KT, 128], fp8, kind="ExternalInput")
    wql_d = nc.dram_tensor("wql", [128, HQL, KT, 128], fp8, kind="ExternalInput")
    wkh_d = nc.dram_tensor("wkh", [128, HKVL, KT, 128], fp8, kind="ExternalInput")
    wkl_d = nc.dram_tensor("wkl", [128, HKVL, KT, 128], fp8, kind="ExternalInput")
    wvh_d = nc.dram_tensor("wvh", [128, HKVL, KT, 128], fp8, kind="ExternalInput")
    wvl_d = nc.dram_tensor("wvl", [128, HKVL, KT, 128], fp8, kind="ExternalInput")
    NE = EOUT // 128
    wph_d = nc.dram_tensor("wph", [128, NE, HQL, 128], fp8, kind="ExternalInput")
    wpl_d = nc.dram_tensor("wpl", [128, NE, HQL, 128], fp8, kind="ExternalInput")
    cos_d = nc.dram_tensor("cos", [128, T], b16, kind="ExternalInput")
    sin_d = nc.dram_tensor("rsin", [64, T], b16, kind="ExternalInput")
    tri_d = nc.dram_tensor("tri", [128, 128], f32, kind="ExternalInput")
    id_d = nc.dram_tensor("ident", [128, 128], b16, kind="ExternalInput")
    yt_d = nc.dram_tensor("yt", [EOUT, T], f32, kind="ExternalOutput")

    with tile.TileContext(nc) as tc:
        with tc.tile_pool(name="glob", bufs=1) as glob:
            cos_sb = glob.tile([128, T], b16)
            sin_sb = glob.tile([64, T], b16)
            tri_sb = glob.tile([128, 128], f32)
            id_sb = glob.tile([128, 128], b16)

            qT = glob.tile([128, HQL, T], b16)       # rope(q)^T per head
            kT = glob.tile([128, HKVL, T], b16)      # rope(k)^T per head
            vON = glob.tile([128, HKVL, NKT, 129], b16)  # [tok, dv | 1]
            nc.vector.memset(vON[:, :, :, 128:129], 1.0)

            # ---------------- Phase A: projections -------------------------
            with tc.tile_pool(name="xt", bufs=1) as xtp, \
                 tc.tile_pool(name="wld", bufs=8) as wld, \
                 tc.tile_pool(name="rtmp", bufs=1) as rtmp, \
                 tc.tile_pool(name="vtmp", bufs=1) as vtmp, \
                 tc.tile_pool(name="psA", bufs=7, space="PSUM") as psA, \
                 tc.tile_pool(name="psT", bufs=1, space="PSUM") as psT:

                KH = KT // 2  # weight half-slab depth (16 tiles)

                def load_w(wh_d_, wl_d_, m):
                    """Returns f(prod, kp) -> [128, 2, 128] AP of the k-pair.
                    prod 0 -> hi weights, 1 -> lo weights."""
                    slabs = {}
                    for key, w_d_ in (("h", wh_d_), ("l", wl_d_)):
                        wa = wld.tile([128, KH, 128], fp8, tag="w",
                                      name=f"wa{key}")
                        nc.sync.dma_start(out=wa[:], in_=w_d_[:, m, 0:KH, :])
                        wb = wld.tile([128, KH, 128], fp8, tag="w",
                                      name=f"wb{key}")
                        nc.sync.dma_start(out=wb[:], in_=w_d_[:, m, KH:KT, :])
                        slabs[key] = (wa, wb)

                    def get(key, kp):
                        wa, wb = slabs[key]
                        k0 = 2 * kp
                        if k0 < KH:
                            return wa[:, k0:k0 + 2, :]
                        return wb[:, k0 - KH:k0 - KH + 2, :]
                    return get

                # DMA issue order tracks first use: the m0/m1 hi a-slabs and
                # first x tiles land first (first matmul ~2.5us in); lo
                # a-slabs early because passes 1+2 run interleaved per
                # k-pair (pass 2 reuses the same xh tiles); big rope tables
                # follow the b-slabs (first rope is ~40us in); xl last.
                xh_sb = xtp.tile([128, KT, T], fp8)
                xl_sb = xtp.tile([128, KT, T], fp8)

                def load_slab(w_d_, m, half):
                    w = wld.tile([128, KH, 128], fp8, tag="w")
                    lo, hi = (0, KH) if half == 0 else (KH, KT)
                    nc.sync.dma_start(out=w[:], in_=w_d_[:, m, lo:hi, :])
                    return w

                wa0h = load_slab(wqh_d, 0, 0)
                nc.sync.dma_start(out=xh_sb[:, 0, :], in_=xh_d[:, 0, :])
                nc.sync.dma_start(out=xh_sb[:, 1, :], in_=xh_d[:, 1, :])
                wa1h = load_slab(wqh_d, 1, 0)
                wa0l = load_slab(wql_d, 0, 0)
                wa1l = load_slab(wql_d, 1, 0)
                for a in range(2, KT):
                    nc.sync.dma_start(out=xh_sb[:, a, :], in_=xh_d[:, a, :])
                    if a == 10:  # b-halves needed from kp=8
                        wb0h = load_slab(wqh_d, 0, 1)
                        wb1h = load_slab(wqh_d, 1, 1)
                        wb0l = load_slab(wql_d, 0, 1)
                        wb1l = load_slab(wql_d, 1, 1)
                nc.sync.dma_start(out=cos_sb[:], in_=cos_d[:])
                nc.sync.dma_start(out=sin_sb[:], in_=sin_d[:])
                nc.sync.dma_start(out=tri_sb[:], in_=tri_d[:])
                nc.sync.dma_start(out=id_sb[:], in_=id_d[:])
                for a in range(KT):
                    nc.sync.dma_start(out=xl_sb[:, a, :], in_=xl_d[:, a, :])

                def mk_wfn(wa, wb, wal, wbl):
                    slabs = {"h": (wa, wb), "l": (wal, wbl)}

                    def get(key, kp):
                        a, b = slabs[key]
                        k0 = 2 * kp
                        if k0 < KH:
                            return a[:, k0:k0 + 2, :]
                        return b[:, k0 - KH:k0 - KH + 2, :]
                    return get

                def rope_evict(ps, dst, c):
                    # dst = ps * cos + rot64(ps) * sin  (bf16 out);
                    # rot[0:64] = -ps[64:128], rot[64:128] = ps[0:64]
                    cs = slice(512 * c, 512 * (c + 1))
                    t1 = rtmp.tile([128, 512], f32, tag="t1")
                    nc.vector.scalar_tensor_tensor(
                        t1[0:64, :], ps[64:128, :], -1.0, sin_sb[:, cs],
                        op0=mult, op1=mult)
                    nc.vector.tensor_tensor(t1[64:128, :], ps[0:64, :],
                                            sin_sb[:, cs], mult)
                    t2 = rtmp.tile([128, 512], f32, tag="t2")
                    nc.vector.tensor_tensor(t2[:], ps[:], cos_sb[:, cs], mult)
                    nc.vector.tensor_tensor(dst, t2[:], t1[:], add)

                # (prod, x-operand) sequence for the compensated product:
                #   x_hi@w_hi + x_hi@w_lo + x_lo@w_hi
                def dr_chain(ps, wfn, c, kp_order=None):
                    cs = slice(512 * c, 512 * (c + 1))
                    steps = [("h", xh_sb), ("l", xh_sb), ("h", xl_sb)]
                    n = 0
                    for si, (wkey, xsb) in enumerate(steps):
                        for kp in range(KP):
                            nc.tensor.matmul(
                                ps[:], lhsT=wfn(wkey, kp),
                                rhs=xsb[:, 2 * kp:2 * kp + 2, cs],
                                start=(n == 0), stop=(n == 3 * KP - 1),
                                perf_mode=DR)
                            n += 1

                # Startup ramp: q-heads 0+1 run kp-outer, interleaved, over 8
                # live psums so PE issues 8 matmuls per freshly-landed x tile
                # and tracks the DMA (pass 1 follows xh, pass 3 follows xl).
                # The last pass runs unit-major so early units stop (and
                # their rope evicts start on DVE) while PE finishes the rest.
                units = [(0, c) for c in range(NCH)] + \
                        [(1, c) for c in range(NCH)]
                wfns = {0: mk_wfn(wa0h, wb0h, wa0l, wb0l),
                        1: mk_wfn(wa1h, wb1h, wa1l, wb1l)}
                pss = {u: psA.tile([128, 512], f32, tag="pj",
                                   name=f"pj{u[0]}_{u[1]}")
                       for u in units[:-1]}
                pss[units[-1]] = psT.tile([128, 512], f32, tag="tr",
                                          name="pj8")
                # passes 1+2 interleaved per k-pair: both read the same two
                # xh tiles, so PE does 16 matmuls per 2-tile DMA landing and
                # stays ahead of the x stream.
                for kp in range(KP):
                    for wkey in ("h", "l"):
                        for (m, c) in units:
                            nc.tensor.matmul(
                                pss[(m, c)][:], lhsT=wfns[m](wkey, kp),
                                rhs=xh_sb[:, 2 * kp:2 * kp + 2,
                                          512 * c:512 * (c + 1)],
                                start=(wkey == "h" and kp == 0), stop=False,
                                perf_mode=DR)
                for (m, c) in units:
                    for kp in range(KP):
                        nc.tensor.matmul(
                            pss[(m, c)][:], lhsT=wfns[m]("h", kp),
                            rhs=xl_sb[:, 2 * kp:2 * kp + 2,
                                      512 * c:512 * (c + 1)],
                            start=False, stop=(kp == KP - 1),
                            perf_mode=DR)
                    rope_evict(pss[(m, c)], qT[:, m, 512 * c:512 * (c + 1)], c)

                # remaining projections: q-heads 2-7, k-heads, then V heads.
                # Next head's weight slabs are prefetched before the current
                # head's chunk chains are issued (ring bufs sized to hold
                # current 4 + next 4 slabs).
                heads = [("r", qT, wqh_d, wql_d, m) for m in range(2, HQL)] + \
                        [("r", kT, wkh_d, wkl_d, m) for m in range(HKVL)] + \
                        [("v", None, wvh_d, wvl_d, m) for m in range(HKVL)]
                wcur = load_w(heads[0][2], heads[0][3], heads[0][4])
                for i, (kind, dst, wh_d_, wl_d_, m) in enumerate(heads):
                    wnxt = (load_w(heads[i + 1][2], heads[i + 1][3],
                                   heads[i + 1][4])
                            if i + 1 < len(heads) else None)
                    for c in range(NCH):
                        ps = psA.tile([128, 512], f32, tag="pj")
                        dr_chain(ps, wcur, c)
                        if kind == "r":
                            rope_evict(ps, dst[:, m, 512 * c:512 * (c + 1)],
                                       c)
                        else:
                            # V: v^T psum -> (1/SW) sbuf -> PE transpose
                            vt = vtmp.tile([128, 512], b16, tag="vt")
                            nc.scalar.activation(vt[:], ps[:], Copy,
                                                 scale=1.0 / SW)
                            pt = psT.tile([128, 512], b16, tag="tr")
                            for s in range(4):
                                nc.tensor.transpose(
                                    pt[:, 128 * s:128 * (s + 1)],
                                    vt[:, 128 * s:128 * (s + 1)], id_sb[:])
                            for s in range(4):
                                nc.scalar.copy(
                                    out=vON[:, m, 4 * c + s, 0:128],
                                    in_=pt[:, 128 * s:128 * (s + 1)])
                    wcur = wnxt

            # ---------------- Phases B + C ---------------------------------
            with tc.tile_pool(name="late", bufs=1) as late, \
                 tc.tile_pool(name="ppool", bufs=10) as ppool, \
                 tc.tile_pool(name="npool", bufs=8) as npool, \
                 tc.tile_pool(name="spool", bufs=4) as spool, \
                 tc.tile_pool(name="psS", bufs=5, space="PSUM") as psS, \
                 tc.tile_pool(name="psP", bufs=1, space="PSUM") as psP, \
                 tc.tile_pool(name="psacc", bufs=1, space="PSUM") as psacc:

                aTh = late.tile([128, HQL, T], fp8)
                aTl = late.tile([128, HQL, T], fp8)
                # wproj packed per e-column-tile so DMA lands in consumption
                # order (proj tiles only need their own e slabs, not all of
                # wproj, when the phase-A -> B transition is DMA-tight)
                wph_sb = late.tile([128, NE, HQL, 128], fp8)
                wpl_sb = late.tile([128, NE, HQL, 128], fp8)
                for e in range(NE):
                    nc.sync.dma_start(out=wph_sb[:, e], in_=wph_d[:, e])
                    nc.sync.dma_start(out=wpl_sb[:, e], in_=wpl_d[:, e])

                # Phases B+C software-pipelined: while attention runs for
                # chunk c, the output projection for chunk c-1 is interleaved
                # between heads (4 e-tiles per head) so PE fills ACT-wait
                # gaps and the output DMA spreads across the whole run.
                def proj_tile(e, c, drain=False):
                    cs = slice(512 * c, 512 * (c + 1))
                    ps = psS.tile([128, 512], f32, tag="s", name="psp")
                    n = 0
                    for wsb, asb in ((wph_sb, aTh), (wpl_sb, aTh),
                                     (wph_sb, aTl)):
                        for hp in range(HQL // 2):
                            nc.tensor.matmul(
                                ps[:], lhsT=wsb[:, e, 2 * hp:2 * hp + 2, :],
                                rhs=asb[:, 2 * hp:2 * hp + 2, cs],
                                start=(n == 0), stop=(n == 3 * HQL // 2 - 1),
                                perf_mode=DR)
                            n += 1
                    yt = ppool.tile([128, 512], f32, tag="yt", name="yt")
                    # in-loop evicts stay off ACT (it paces the exp chain);
                    # the drain has no exps so it alternates
                    if drain and e % 2 == 1:
                        nc.scalar.copy(out=yt[:], in_=ps[:])
                    else:
                        nc.vector.tensor_copy(yt[:], ps[:])
                    nc.sync.dma_start(
                        out=yt_d[128 * e:128 * (e + 1), 512 * c:512 * (c + 1)],
                        in_=yt[:])

                NE = EOUT // 128
                EPH = NE // HQL  # proj e-tiles interleaved per head
                # carry: [(emit_ti, fn)] — work of the PREVIOUS head (tail
                # AV matmuls + normalize, then A^T transposes) deferred into
                # the current head's t-loop so PE never waits on it inline.
                carry = []
                for c in range(NCH):
                    for h in range(HQL):
                        v = h // REP
                        # two AV chains share each accumulator bank: only
                        # the even chain issues start=True — its whole-bank
                        # pending-zero mark also zero-fills the odd chain's
                        # region on first write (HW zero-region semantics)
                        acc2 = [psacc.tile([128, 258], f32, tag=f"acc{i}",
                                           name=f"acc{i}")
                                for i in range(2)]
                        accs = [acc2[s // 2][:, 129 * (s % 2):
                                             129 * (s % 2) + 129]
                                for s in range(4)]
                        n_tk = 4 * c + 4
                        pTs = {}

                        def vmms(t, accs=accs, pTs=pTs, v=v, c=c):
                            j = t - 4 * c
                            for s in range(4):
                                if j > s:
                                    continue
                                # emission order: fulls interleaved with
                                # diags, ending [..., full 4c-1, diag 4c+3]
                                # group flags are bank-level: the even
                                # chain opens the bank (start), the odd
                                # chain's last op closes it (stop) — the
                                # odd chain's region is zero-filled by the
                                # bank-wide pending-zero of the start.
                                stop_t = (s if c == 0 else
                                          (4 * c + 3 if s == 3 else
                                           4 * c - 1))
                                nc.tensor.matmul(
                                    accs[s],
                                    lhsT=pTs[t][:, 128 * s:128 * (s + 1)],
                                    rhs=vON[:, v, t, :],
                                    start=(t == 0 and s % 2 == 0),
                                    stop=(t == stop_t and s % 2 == 1))

                        # proj tiles of the previous chunk, interleaved into
                        # the t-loop (own psum bank) to fill ACT-paced gaps
                        pe_list = (list(range(EPH * h, EPH * (h + 1)))
                                   if c > 0 else [])
                        D = 4 if c > 0 else 3
                        # diag tiles spread through the loop (one after
                        # every c full tiles): their ACT exp work and DVE
                        # tri-adds never bunch at the tail, and the loop
                        # ends on the smallest (128-col) exp.
                        t_seq = []
                        for i in range(4):
                            t_seq += list(range(c * i, c * (i + 1)))
                            t_seq.append(4 * c + i)
                        for ti, t in enumerate(t_seq):
                            j = t - 4 * c  # >= 0 on diagonal-group tiles
                            col0 = 128 * j if j > 0 else 0
                            ps = psS.tile([128, 512], f32, tag="s")
                            nc.tensor.matmul(
                                ps[:, col0:512],
                                lhsT=kT[:, v, 128 * t:128 * (t + 1)],
                                rhs=qT[:, h, 512 * c + col0:512 * (c + 1)],
                                start=True, stop=True)
                            while carry and carry[0][0] <= ti:
                                carry.pop(0)[1]()
                            if j >= 0:
                                nc.vector.tensor_tensor(
                                    ps[:, 128 * j:128 * (j + 1)],
                                    ps[:, 128 * j:128 * (j + 1)],
                                    tri_sb[:], add)
                            pT = ppool.tile([128, 512], b16, tag="pT")
                            nc.scalar.activation(
                                pT[:, col0:512], ps[:, col0:512], Exp,
                                scale=scale)
                            pTs[t] = pT
                            if ti >= D:
                                vmms(t_seq[ti - D])
                            # proj tiles read ALL heads' aT of chunk c-1;
                            # at h==0 the previous chunk's last head's
                            # transposes are only emitted at ti==6, so its
                            # proj tiles must come after (reads emitted
                            # before writes get no dependency edge)
                            if pe_list and (h > 0 or ti >= 6) and \
                               (ti + 1) * EPH // (n_tk + 3) > \
                               ti * EPH // (n_tk + 3):
                                proj_tile(pe_list.pop(0), c - 1)
                        for e in pe_list:
                            proj_tile(e, c - 1)

                        # Package this head's tail: the last D AV matmuls
                        # (their exps are still draining on ACT) plus the
                        # normalize chain; and, later, the A^T transposes.
                        # Both run inside the NEXT head's t-loop.
                        holder = []

                        def make_tail(vmms=vmms, accs=accs, n_tk=n_tk, D=D,
                                      holder=holder, t_seq=tuple(t_seq)):
                            def emit():
                                for t in t_seq[max(0, n_tk - D):]:
                                    vmms(t)
                                # batched normalize: 4 s-blocks land in one
                                # [128,512] bf16 tile; the fp8 hi/lo split
                                # happens AFTER the transpose (identical
                                # math, and bf16 PE transposes are legal
                                # where fp8 ones need stride-2 outputs)
                                an = npool.tile([128, 512], b16, tag="an")
                                for s in range(4):
                                    rec = spool.tile([128, 1], f32,
                                                     tag="rec")
                                    nc.vector.reciprocal(
                                        rec[:], accs[s][:, 128:129])
                                    nc.vector.tensor_scalar_mul(
                                        an[:, 128 * s:128 * (s + 1)],
                                        accs[s][:, 0:128], rec[:])
                                holder.append(an)
                            return emit

                        def make_tr(holder=holder, h=h, c=c):
                            def emit():
                                pt = psP.tile([128, 512], b16, tag="p8",
                                              name="pt8")
                                an = holder[0]
                                for s in range(4):
                                    nc.tensor.transpose(
                                        pt[:, 128 * s:128 * (s + 1)],
                                        an[:, 128 * s:128 * (s + 1)],
                                        id_sb[:])
                                cs = slice(512 * c, 512 * (c + 1))
                                # post-transpose hi/lo split: hi on ACT,
                                # lo = pt - hi on DVE
                                nc.scalar.copy(out=aTh[:, h, cs],
                                               in_=pt[:])
                                nc.vector.tensor_tensor(
                                    aTl[:, h, cs], pt[:], aTh[:, h, cs],
                                    sub)
                            return emit

                        for _, fn in carry:  # flush any unemitted leftovers
                            fn()
                        # emit points must fit inside the NEXT iteration's
                        # t-loop (n_tk=4 when it is a c==0 head)
                        nxt_c0 = (c == 0 and h < HQL - 1)
                        carry = [(1, make_tail()),
                                 (3 if nxt_c0 else 6, make_tr())]

                for _, fn in carry:
                    fn()
                # drain: projection of the last chunk through the 3-bank ring
                for e in range(NE):
                    proj_tile(e, NCH - 1)

    nc.compile()
    return nc


def _rope_tables(T=T):
    j = np.arange(64, dtype=np.float64)
    inv_freq = 1.0 / (BASE_FREQ ** (2.0 * j / HD))
    t = np.arange(T, dtype=np.float64)
    fr = t[:, None] * inv_freq[None, :]          # [T, 64]
    cos = np.cos(fr) / SW                        # fold 1/SW (fp8 w scaling)
    sin = np.sin(fr) / SW
    cos_tbl = np.concatenate([cos, cos], axis=1).T    # [128, T]
    sin_tbl = sin.T                                   # [64, T]
    return cos_tbl.astype(bf16), sin_tbl.astype(bf16)


def _hilo(a):
    """fp8 e4m3 hi/lo split of a float32 array."""
    h = a.astype(f8e4)
    l = (a - h.astype(np.float32)).astype(f8e4)
    return h, l


def _pack_w(w):
    """[KE, M] -> [128, M//128, KE//128, 128]: w_l[p, m, a, j] = w[128a+p, 128m+j]."""
    KE, M = w.shape
    return np.ascontiguousarray(
        w.reshape(KE // 128, 128, M // 128, 128).transpose(1, 2, 0, 3))


def prep_core_inputs(x, wq, wk, wv, wproj):
    cos_tbl, rsin_tbl = _rope_tables()
    tri = np.where(np.arange(128)[None, :] >= np.arange(128)[:, None],
                   0.0, NEG).astype(np.float32)
    ident = np.eye(128, dtype=bf16)

    # shared fp8 splits (sliced per core below)
    wqh, wql = _hilo(wq * SW)
    wkh, wkl = _hilo(wk * SW)
    wvh, wvl = _hilo(wv * SW)
    wph, wpl = _hilo(wproj * SW)

    xs = []
    for b in range(B):
        xt = np.ascontiguousarray(
            x[b].T.reshape(N_EMBD // 128, 128, T).transpose(1, 0, 2))
        xs.append(_hilo(xt.astype(np.float32)))

    def packp(w, cols):
        # [1024, E] -> [128, NE, HQL, 128]: per e-column-tile slabs
        return np.ascontiguousarray(
            w[cols, :].reshape(HQL, 128, N_EMBD // 128, 128)
            .transpose(1, 2, 0, 3))

    in_maps = []
    for ci in range(N_CORES):
        b, g = divmod(ci, TPG)
        qcols = slice(g * HQL * HD, (g + 1) * HQL * HD)
        kvcols = slice(g * HKVL * HD, (g + 1) * HKVL * HD)
        in_maps.append({
            "xh": xs[b][0], "xl": xs[b][1],
            "wqh": _pack_w(wqh[:, qcols]), "wql": _pack_w(wql[:, qcols]),
            "wkh": _pack_w(wkh[:, kvcols]), "wkl": _pack_w(wkl[:, kvcols]),
            "wvh": _pack_w(wvh[:, kvcols]), "wvl": _pack_w(wvl[:, kvcols]),
            "wph": packp(wph, qcols), "wpl": packp(wpl, qcols),
            "cos": cos_tbl, "rsin": rsin_tbl, "tri": tri, "ident": ident,
        })
    return in_maps


_NC_CACHE = {}


def _get_nc():
    if "nc" not in _NC_CACHE:
        _NC_CACHE["nc"] = build_nc()
    return _NC_CACHE["nc"]


def _get_runner():
    """Cached sharded-jit executor over the 8 cores (no donation, so the
    compiled executable is reusable across calls)."""
    if "runner" in _NC_CACHE:
        return _NC_CACHE["runner"]
    import jax
    from jax.sharding import Mesh, PartitionSpec, NamedSharding
    from jax.experimental.shard_map import shard_map
    from concourse import mybir
    from concourse.bass2jax import (_bass_exec_p, install_neuronx_cc_hook,
                                    partition_id_tensor)

    nc = _get_nc()
    install_neuronx_cc_hook()
    pname = nc.partition_id_tensor.name if nc.partition_id_tensor else None
    in_names, out_names, out_avals, zero_shapes = [], [], [], []
    for alloc in nc.m.functions[0].allocations:
        if not isinstance(alloc, mybir.MemoryLocationSet):
            continue
        name = alloc.memorylocations[0].name
        if alloc.kind == "ExternalInput":
            if name != pname:
                in_names.append(name)
        elif alloc.kind == "ExternalOutput":
            out_names.append(name)
            shape = tuple(alloc.tensor_shape)
            dtype = mybir.dt.np(alloc.dtype)
            out_avals.append(jax.core.ShapedArray(shape, dtype))
            zero_shapes.append((shape, dtype))
    all_names = in_names + out_names + ([pname] if pname else [])

    def _body(*args):
        operands = list(args)
        if pname:
            operands.append(partition_id_tensor())
        return tuple(_bass_exec_p.bind(
            *operands, out_avals=tuple(out_avals), in_names=tuple(all_names),
            out_names=tuple(out_names), lowering_input_output_aliases=(),
            sim_require_finite=True, sim_require_nnan=True, nc=nc))

    devices = jax.devices()[:N_CORES]
    mesh = Mesh(np.asarray(devices), ("core",))
    nin = len(in_names) + len(out_names)
    sharded = jax.jit(
        shard_map(_body, mesh=mesh, in_specs=(PartitionSpec("core"),) * nin,
                  out_specs=(PartitionSpec("core"),) * len(out_names),
                  check_rep=False),
        keep_unused=True)
    sh = NamedSharding(mesh, PartitionSpec("core"))
    zeros = [jax.device_put(
        np.zeros((N_CORES * s[0], *s[1:]), dt), sh)
        for s, dt in zero_shapes]

    def run(in_maps):
        concat = [np.concatenate([m[n] for m in in_maps], axis=0)
                  for n in in_names]
        dev_in = [jax.device_put(a, sh) for a in concat]
        outs = sharded(*dev_in, *zeros)
        jax.block_until_ready(outs)
        return [
            {n: np.asarray(outs[i]).reshape(N_CORES, *out_avals[i].shape)[ci]
             for i, n in enumerate(out_names)}
            for ci in range(N_CORES)]

    _NC_CACHE["runner"] = run
    return run


def kernel(x, wq, wk, wv, wproj):
    in_maps = prep_core_inputs(np.asarray(x, dtype=np.float32),
                               np.asarray(wq, dtype=np.float32),
                               np.asarray(wk, dtype=np.float32),
                               np.asarray(wv, dtype=np.float32),
                               np.asarray(wproj, dtype=np.float32))
    results = _get_runner()(in_maps)
    y = np.empty((B, T, N_EMBD), dtype=np.float32)
    for b in range(B):
        acc = results[b * TPG]["yt"].copy()
        for g in range(1, TPG):
            acc += results[b * TPG + g]["yt"]
        y[b] = acc.T / SW
    return y


if __name__ == "__main__":
    rng = np.random.default_rng(0)
    x = rng.standard_normal((B, T, N_EMBD), dtype=np.float32)
    wq_ = (rng.standard_normal((N_EMBD, N_EMBD), dtype=np.float32) * 0.02)
    wk_ = (rng.standard_normal((N_EMBD, HKV * HD), dtype=np.float32) * 0.02)
    wv_ = (rng.standard_normal((N_EMBD, HKV * HD), dtype=np.float32) * 0.02)
    wp_ = (rng.standard_normal((N_EMBD, N_EMBD), dtype=np.float32) * 0.02)
    y = kernel(x, wq_, wk_, wv_, wp_)
    print("out", y.shape, y.dtype, float(np.abs(y).max()))


# revision 73
# speedup vs baseline: 1.2763x; 1.0224x over previous
"""Trainium2 Bass kernel for causal self-attention (GQA + RoPE).

Problem: B=2, T=2048, n_embd=4096, HQ=32 q-heads, HKV=8 kv-heads, HD=128.
  q = rope(x @ wq), k = rope(x @ wk), v = x @ wv
  y = causal_softmax(q k^T / sqrt(HD)) v @ wproj

Sharding (8 cores): core = (b, g), b in {0,1} batch, g in {0..3} head-group.
Each core handles 8 q-heads / 2 kv-heads of one batch sample:
  - wq/wk/wv column-sharded, wproj row-sharded (tensor parallel over heads)
  - final reduce (sum of 4 partial y per batch) done on host in fp32.

Per-core device program:
  A) projections in COMPENSATED fp8 (e4m3 hi+lo splits of x and w, x64
     weight scaling folded into the rope tables / V eviction): per
     (head, chunk) one PSUM accumulates 48 DoubleRow matmuls
     (3 products x 16 k-tile pairs, 256-deep contraction each), then
     fused RoPE evict (bf16 out).  V^T is PE-transposed into V[tok, dv]
     with a ones column appended.
  B) attention per (head, 512-token q-chunk) in bf16: S^T =
     K-block^T-matmul(Q^T), causal tri mask on diag blocks, ACT exp ->
     P^T (bf16), then out[tq,129] += P^T-block.T @ [V|1] (rowsum rides
     in col 128), normalize, split into fp8 hi+lo, PE-transpose ->
     A^T_hi/A^T_lo [dv, t].
  C) y^T = compensated fp8 DoubleRow over head pairs:
     3 products x 4 head-pairs per e-tile, fp32 eviction, DMA out.
"""
import sys

if "/opt/trn_rl_repo" not in sys.path:
    sys.path.insert(0, "/opt/trn_rl_repo")

import math
import numpy as np
import ml_dtypes

B, T, N_EMBD = 2, 2048, 4096
HQ, HKV = 32, 8
HD = 128
N_CORES = 8
TPG = 4                      # tensor-parallel groups per batch
HQL, HKVL = HQ // TPG, HKV // TPG   # 8 q-heads, 2 kv-heads per core
SCALE = 1.0 / math.sqrt(HD)
BASE_FREQ = 10000.0
NEG = -1e30
SW = 64.0                    # fp8 weight pre-scale

bf16 = ml_dtypes.bfloat16
f8e4 = ml_dtypes.float8_e4m3


def build_nc(T=T, KE=N_EMBD, HQL=HQL, HKVL=HKVL, EOUT=N_EMBD, scale=SCALE):
    """Build the per-core Bass program. All shapes hardcoded at trace time."""
    import concourse.tile as tile
    from concourse import bacc, mybir

    f32 = mybir.dt.float32
    b16 = mybir.dt.bfloat16
    fp8 = mybir.dt.float8e4
    Exp = mybir.ActivationFunctionType.Exp
    Copy = mybir.ActivationFunctionType.Copy
    DR = mybir.MatmulPerfMode.DoubleRow
    mult = mybir.AluOpType.mult
    add = mybir.AluOpType.add
    sub = mybir.AluOpType.subtract

    KT = KE // 128          # contraction tiles for projections
    KP = KT // 2            # DoubleRow k-tile pairs
    NKT = T // 128          # token tiles
    NCH = T // 512          # token chunks
    REP = HQL // HKVL

    nc = bacc.Bacc("TRN2", target_bir_lowering=False)

    xh_d = nc.dram_tensor("xh", [128, KT, T], fp8, kind="ExternalInput")
    xl_d = nc.dram_tensor("xl", [128, KT, T], fp8, kind="ExternalInput")
    wqh_d = nc.dram_tensor("wqh", [128, HQL, KT, 128], fp8, kind="ExternalInput")
    wql_d = nc.dram_tensor("wql", [128, HQL, KT, 128], fp8, kind="ExternalInput")
    wkh_d = nc.dram_tensor("wkh", [128, HKVL, KT, 128], fp8, kind="ExternalInput")
    wkl_d = nc.dram_tensor("wkl", [128, HKVL, KT, 128], fp8, kind="ExternalInput")
    wvh_d = nc.dram_tensor("wvh", [128, HKVL, KT, 128], fp8, kind="ExternalInput")
    wvl_d = nc.dram_tensor("wvl", [128, HKVL, KT, 128], fp8, kind="ExternalInput")
    NE = EOUT // 128
    wph_d = nc.dram_tensor("wph", [128, NE, HQL, 128], fp8, kind="ExternalInput")
    wpl_d = nc.dram_tensor("wpl", [128, NE, HQL, 128], fp8, kind="ExternalInput")
    cos_d = nc.dram_tensor("cos", [128, T], b16, kind="ExternalInput")
    sin_d = nc.dram_tensor("rsin", [64, T], b16, kind="ExternalInput")
    tri_d = nc.dram_tensor("tri", [128, 128], f32, kind="ExternalInput")
    id_d = nc.dram_tensor("ident", [128, 128], b16, kind="ExternalInput")
    yt_d = nc.dram_tensor("yt", [EOUT, T], f32, kind="ExternalOutput")

    with tile.TileContext(nc) as tc:
        with tc.tile_pool(name="glob", bufs=1) as glob:
            cos_sb = glob.tile([128, T], b16)
            sin_sb = glob.tile([64, T], b16)
            tri_sb = glob.tile([128, 128], f32)
            id_sb = glob.tile([128, 128], b16)

            qT = glob.tile([128, HQL, T], b16)       # rope(q)^T per head
            kT = glob.tile([128, HKVL, T], b16)      # rope(k)^T per head
            vON = glob.tile([128, HKVL, NKT, 129], b16)  # [tok, dv | 1]
            nc.vector.memset(vON[:, :, :, 128:129], 1.0)
            # exp bias for the fp8-P path (max exp-arg 11.41; e^5.41 < 240)
            pb_sb = glob.tile([128, 1], f32)
            nc.vector.memset(pb_sb[:], -6.0)

            # ---------------- Phase A: projections -------------------------
            with tc.tile_pool(name="xt", bufs=1) as xtp, \
                 tc.tile_pool(name="wld", bufs=8) as wld, \
                 tc.tile_pool(name="rtmp", bufs=1) as rtmp, \
                 tc.tile_pool(name="vtmp", bufs=1) as vtmp, \
                 tc.tile_pool(name="psA", bufs=7, space="PSUM") as psA, \
                 tc.tile_pool(name="psT", bufs=1, space="PSUM") as psT:

                KH = KT // 2  # weight half-slab depth (16 tiles)

                def load_w(wh_d_, wl_d_, m):
                    """Returns f(prod, kp) -> [128, 2, 128] AP of the k-pair.
                    prod 0 -> hi weights, 1 -> lo weights."""
                    slabs = {}
                    for key, w_d_ in (("h", wh_d_), ("l", wl_d_)):
                        wa = wld.tile([128, KH, 128], fp8, tag="w",
                                      name=f"wa{key}")
                        nc.sync.dma_start(out=wa[:], in_=w_d_[:, m, 0:KH, :])
                        wb = wld.tile([128, KH, 128], fp8, tag="w",
                                      name=f"wb{key}")
                        nc.sync.dma_start(out=wb[:], in_=w_d_[:, m, KH:KT, :])
                        slabs[key] = (wa, wb)

                    def get(key, kp):
                        wa, wb = slabs[key]
                        k0 = 2 * kp
                        if k0 < KH:
                            return wa[:, k0:k0 + 2, :]
                        return wb[:, k0 - KH:k0 - KH + 2, :]
                    return get

                # DMA issue order tracks first use: the m0/m1 hi a-slabs and
                # first x tiles land first (first matmul ~2.5us in); lo
                # a-slabs early because passes 1+2 run interleaved per
                # k-pair (pass 2 reuses the same xh tiles); big rope tables
                # follow the b-slabs (first rope is ~40us in); xl last.
                xh_sb = xtp.tile([128, KT, T], fp8)
                xl_sb = xtp.tile([128, KT, T], fp8)

                def load_slab(w_d_, m, half):
                    w = wld.tile([128, KH, 128], fp8, tag="w")
                    lo, hi = (0, KH) if half == 0 else (KH, KT)
                    nc.sync.dma_start(out=w[:], in_=w_d_[:, m, lo:hi, :])
                    return w

                wa0h = load_slab(wqh_d, 0, 0)
                nc.sync.dma_start(out=xh_sb[:, 0, :], in_=xh_d[:, 0, :])
                nc.sync.dma_start(out=xh_sb[:, 1, :], in_=xh_d[:, 1, :])
                wa1h = load_slab(wqh_d, 1, 0)
                wa0l = load_slab(wql_d, 0, 0)
                wa1l = load_slab(wql_d, 1, 0)
                for a in range(2, KT):
                    nc.sync.dma_start(out=xh_sb[:, a, :], in_=xh_d[:, a, :])
                    if a == 10:  # b-halves needed from kp=8
                        wb0h = load_slab(wqh_d, 0, 1)
                        wb1h = load_slab(wqh_d, 1, 1)
                        wb0l = load_slab(wql_d, 0, 1)
                        wb1l = load_slab(wql_d, 1, 1)
                nc.sync.dma_start(out=cos_sb[:], in_=cos_d[:])
                nc.sync.dma_start(out=sin_sb[:], in_=sin_d[:])
                nc.sync.dma_start(out=tri_sb[:], in_=tri_d[:])
                nc.sync.dma_start(out=id_sb[:], in_=id_d[:])
                for a in range(KT):
                    nc.sync.dma_start(out=xl_sb[:, a, :], in_=xl_d[:, a, :])

                def mk_wfn(wa, wb, wal, wbl):
                    slabs = {"h": (wa, wb), "l": (wal, wbl)}

                    def get(key, kp):
                        a, b = slabs[key]
                        k0 = 2 * kp
                        if k0 < KH:
                            return a[:, k0:k0 + 2, :]
                        return b[:, k0 - KH:k0 - KH + 2, :]
                    return get

                def rope_evict(ps, dst, c):
                    # dst = ps * cos + rot64(ps) * sin  (bf16 out);
                    # rot[0:64] = -ps[64:128], rot[64:128] = ps[0:64]
                    cs = slice(512 * c, 512 * (c + 1))
                    t1 = rtmp.tile([128, 512], f32, tag="t1")
                    nc.vector.scalar_tensor_tensor(
                        t1[0:64, :], ps[64:128, :], -1.0, sin_sb[:, cs],
                        op0=mult, op1=mult)
                    nc.vector.tensor_tensor(t1[64:128, :], ps[0:64, :],
                                            sin_sb[:, cs], mult)
                    t2 = rtmp.tile([128, 512], f32, tag="t2")
                    nc.vector.tensor_tensor(t2[:], ps[:], cos_sb[:, cs], mult)
                    nc.vector.tensor_tensor(dst, t2[:], t1[:], add)

                # (prod, x-operand) sequence for the compensated product:
                #   x_hi@w_hi + x_hi@w_lo + x_lo@w_hi
                def dr_chain(ps, wfn, c, kp_order=None):
                    cs = slice(512 * c, 512 * (c + 1))
                    steps = [("h", xh_sb), ("l", xh_sb), ("h", xl_sb)]
                    n = 0
                    for si, (wkey, xsb) in enumerate(steps):
                        for kp in range(KP):
                            nc.tensor.matmul(
                                ps[:], lhsT=wfn(wkey, kp),
                                rhs=xsb[:, 2 * kp:2 * kp + 2, cs],
                                start=(n == 0), stop=(n == 3 * KP - 1),
                                perf_mode=DR)
                            n += 1

                # Startup ramp: q-heads 0+1 run kp-outer, interleaved, over 8
                # live psums so PE issues 8 matmuls per freshly-landed x tile
                # and tracks the DMA (pass 1 follows xh, pass 3 follows xl).
                # The last pass runs unit-major so early units stop (and
                # their rope evicts start on DVE) while PE finishes the rest.
                units = [(0, c) for c in range(NCH)] + \
                        [(1, c) for c in range(NCH)]
                wfns = {0: mk_wfn(wa0h, wb0h, wa0l, wb0l),
                        1: mk_wfn(wa1h, wb1h, wa1l, wb1l)}
                pss = {u: psA.tile([128, 512], f32, tag="pj",
                                   name=f"pj{u[0]}_{u[1]}")
                       for u in units[:-1]}
                pss[units[-1]] = psT.tile([128, 512], f32, tag="tr",
                                          name="pj8")
                # passes 1+2 interleaved per k-pair: both read the same two
                # xh tiles, so PE does 16 matmuls per 2-tile DMA landing and
                # stays ahead of the x stream.
                for kp in range(KP):
                    for wkey in ("h", "l"):
                        for (m, c) in units:
                            nc.tensor.matmul(
                                pss[(m, c)][:], lhsT=wfns[m](wkey, kp),
                                rhs=xh_sb[:, 2 * kp:2 * kp + 2,
                                          512 * c:512 * (c + 1)],
                                start=(wkey == "h" and kp == 0), stop=False,
                                perf_mode=DR)
                for (m, c) in units:
                    for kp in range(KP):
                        nc.tensor.matmul(
                            pss[(m, c)][:], lhsT=wfns[m]("h", kp),
                            rhs=xl_sb[:, 2 * kp:2 * kp + 2,
                                      512 * c:512 * (c + 1)],
                            start=False, stop=(kp == KP - 1),
                            perf_mode=DR)
                    rope_evict(pss[(m, c)], qT[:, m, 512 * c:512 * (c + 1)], c)

                # remaining projections: q-heads 2-7, k-heads, then V heads.
                # Next head's weight slabs are prefetched before the current
                # head's chunk chains are issued (ring bufs sized to hold
                # current 4 + next 4 slabs).
                heads = [("r", qT, wqh_d, wql_d, m) for m in range(2, HQL)] + \
                        [("r", kT, wkh_d, wkl_d, m) for m in range(HKVL)] + \
                        [("v", None, wvh_d, wvl_d, m) for m in range(HKVL)]
                wcur = load_w(heads[0][2], heads[0][3], heads[0][4])
                for i, (kind, dst, wh_d_, wl_d_, m) in enumerate(heads):
                    wnxt = (load_w(heads[i + 1][2], heads[i + 1][3],
                                   heads[i + 1][4])
                            if i + 1 < len(heads) else None)
                    for c in range(NCH):
                        ps = psA.tile([128, 512], f32, tag="pj")
                        dr_chain(ps, wcur, c)
                        if kind == "r":
                            rope_evict(ps, dst[:, m, 512 * c:512 * (c + 1)],
                                       c)
                        else:
                            # V: v^T psum -> (1/SW) sbuf -> PE transpose
                            vt = vtmp.tile([128, 512], b16, tag="vt")
                            nc.scalar.activation(vt[:], ps[:], Copy,
                                                 scale=1.0 / SW)
                            pt = psT.tile([128, 512], b16, tag="tr")
                            for s in range(4):
                                nc.tensor.transpose(
                                    pt[:, 128 * s:128 * (s + 1)],
                                    vt[:, 128 * s:128 * (s + 1)], id_sb[:])
                            for s in range(4):
                                nc.scalar.copy(
                                    out=vON[:, m, 4 * c + s, 0:128],
                                    in_=pt[:, 128 * s:128 * (s + 1)])
                    wcur = wnxt

            # ---------------- Phases B + C ---------------------------------
            with tc.tile_pool(name="late", bufs=1) as late, \
                 tc.tile_pool(name="ppool", bufs=6) as ppool, \
                 tc.tile_pool(name="p2pool", bufs=2) as p2pool, \
                 tc.tile_pool(name="npool", bufs=8) as npool, \
                 tc.tile_pool(name="spool", bufs=4) as spool, \
                 tc.tile_pool(name="psS", bufs=5, space="PSUM") as psS, \
                 tc.tile_pool(name="psP", bufs=1, space="PSUM") as psP, \
                 tc.tile_pool(name="psacc", bufs=1, space="PSUM") as psacc:

                aTh = late.tile([128, HQL, T], fp8)
                aTl = late.tile([128, HQL, T], fp8)
                # fp8 copy of vON for the DoubleRow-paired AV path (c>=1;
                # converted here where SBUF and DVE both have slack)
                vON8 = late.tile([128, HKVL, NKT, 129], fp8)
                for m in range(HKVL):
                    for hf in range(2):
                        nc.vector.tensor_copy(
                            vON8[:, m, 8 * hf:8 * hf + 8, :],
                            vON[:, m, 8 * hf:8 * hf + 8, :])
                # wproj packed per e-column-tile so DMA lands in consumption
                # order (proj tiles only need their own e slabs, not all of
                # wproj, when the phase-A -> B transition is DMA-tight)
                wph_sb = late.tile([128, NE, HQL, 128], fp8)
                wpl_sb = late.tile([128, NE, HQL, 128], fp8)
                for e in range(NE):
                    nc.sync.dma_start(out=wph_sb[:, e], in_=wph_d[:, e])
                    nc.sync.dma_start(out=wpl_sb[:, e], in_=wpl_d[:, e])

                # Phases B+C software-pipelined: while attention runs for
                # chunk c, the output projection for chunk c-1 is interleaved
                # between heads (4 e-tiles per head) so PE fills ACT-wait
                # gaps and the output DMA spreads across the whole run.
                def proj_tile(e, c, drain=False):
                    cs = slice(512 * c, 512 * (c + 1))
                    ps = psS.tile([128, 512], f32, tag="s", name="psp")
                    n = 0
                    for wsb, asb in ((wph_sb, aTh), (wpl_sb, aTh),
                                     (wph_sb, aTl)):
                        for hp in range(HQL // 2):
                            nc.tensor.matmul(
                                ps[:], lhsT=wsb[:, e, 2 * hp:2 * hp + 2, :],
                                rhs=asb[:, 2 * hp:2 * hp + 2, cs],
                                start=(n == 0), stop=(n == 3 * HQL // 2 - 1),
                                perf_mode=DR)
                            n += 1
                    yt = ppool.tile([128, 512], f32, tag="yt", name="yt")
                    # in-loop evicts stay off ACT (it paces the exp chain);
                    # the drain has no exps so it alternates
                    if drain and e % 2 == 1:
                        nc.scalar.copy(out=yt[:], in_=ps[:])
                    else:
                        nc.vector.tensor_copy(yt[:], ps[:])
                    nc.sync.dma_start(
                        out=yt_d[128 * e:128 * (e + 1), 512 * c:512 * (c + 1)],
                        in_=yt[:])

                NE = EOUT // 128
                EPH = NE // HQL  # proj e-tiles interleaved per head
                # carry: [(emit_ti, fn)] — work of the PREVIOUS head (tail
                # AV matmuls + normalize, then A^T transposes) deferred into
                # the current head's t-loop so PE never waits on it inline.
                carry = []
                for c in range(NCH):
                    for h in range(HQL):
                        v = h // REP
                        # two AV chains share each accumulator bank: only
                        # the even chain issues start=True — its whole-bank
                        # pending-zero mark also zero-fills the odd chain's
                        # region on first write (HW zero-region semantics)
                        acc2 = [psacc.tile([128, 258], f32, tag=f"acc{i}",
                                           name=f"acc{i}")
                                for i in range(2)]
                        accs = [acc2[s // 2][:, 129 * (s % 2):
                                             129 * (s % 2) + 129]
                                for s in range(4)]
                        n_tk = 4 * c + 4
                        pTs = {}
                        pT2 = None
                        if c > 0:
                            # fp8 P for the DoubleRow-paired AV path; the
                            # diag tiles' masked column ranges must be
                            # zeroed by THIS tile generation (inherited
                            # bytes have no cross-generation ordering)
                            pT2 = p2pool.tile([128, NKT, 512], fp8,
                                              tag="p2")
                            for j in range(1, 4):
                                nc.vector.memset(
                                    pT2[:, 4 * c + j, 0:128 * j], 0.0)

                        def vmms(t, accs=accs, pTs=pTs, v=v, c=c):
                            # c==0 single-tile bf16 path
                            j = t - 4 * c
                            for s in range(4):
                                if j > s:
                                    continue
                                nc.tensor.matmul(
                                    accs[s],
                                    lhsT=pTs[t][:, 128 * s:128 * (s + 1)],
                                    rhs=vON[:, v, t, :],
                                    start=(t == 0 and s % 2 == 0),
                                    stop=(t == s and s % 2 == 1))

                        def vmms_pair(p, accs=accs, v=v, c=c, pT2=pT2):
                            # c>=1 DoubleRow path: one instr contracts the
                            # t-pair (2p, 2p+1); masked diag regions in pT2
                            # are zero so they contribute nothing
                            j0 = 2 * p - 4 * c
                            for s in range(4):
                                if j0 > s:  # both tiles fully masked
                                    continue
                                nc.tensor.matmul(
                                    accs[s],
                                    lhsT=pT2[:, 2 * p:2 * p + 2,
                                             128 * s:128 * (s + 1)],
                                    rhs=vON8[:, v, 2 * p:2 * p + 2, :],
                                    start=(p == 0 and s % 2 == 0),
                                    stop=((s == 1 and p == 2 * c - 1) or
                                          (s == 3 and p == 2 * c + 1)),
                                    perf_mode=DR)

                        # proj tiles of the previous chunk, interleaved into
                        # the t-loop (own psum bank) to fill ACT-paced gaps
                        pe_list = (list(range(EPH * h, EPH * (h + 1)))
                                   if c > 0 else [])
                        D = 3
                        # diag tiles spread through the loop (one after
                        # every c full tiles): their ACT exp work and DVE
                        # tri-adds never bunch at the tail, and the loop
                        # ends on the smallest (128-col) exp.
                        t_seq = []
                        for i in range(4):
                            t_seq += list(range(c * i, c * (i + 1)))
                            t_seq.append(4 * c + i)
                        # pair p becomes ready at the position of its later
                        # tile; emit its AV matmuls D positions later
                        ready = {}
                        for ti, t in enumerate(t_seq):
                            ready[t // 2] = ti
                        pair_seq = sorted(ready, key=lambda p: ready[p])
                        pq = list(pair_seq)
                        for ti, t in enumerate(t_seq):
                            j = t - 4 * c  # >= 0 on diagonal-group tiles
                            col0 = 128 * j if j > 0 else 0
                            ps = psS.tile([128, 512], f32, tag="s")
                            nc.tensor.matmul(
                                ps[:, col0:512],
                                lhsT=kT[:, v, 128 * t:128 * (t + 1)],
                                rhs=qT[:, h, 512 * c + col0:512 * (c + 1)],
                                start=True, stop=True)
                            while carry and carry[0][0] <= ti:
                                carry.pop(0)[1]()
                            if j >= 0:
                                nc.vector.tensor_tensor(
                                    ps[:, 128 * j:128 * (j + 1)],
                                    ps[:, 128 * j:128 * (j + 1)],
                                    tri_sb[:], add)
                            if c > 0:
                                nc.scalar.activation(
                                    pT2[:, t, col0:512], ps[:, col0:512],
                                    Exp, scale=scale, bias=pb_sb[:])
                                while pq and ready[pq[0]] <= ti - D:
                                    vmms_pair(pq.pop(0))
                            else:
                                pT = ppool.tile([128, 512], b16, tag="pT")
                                nc.scalar.activation(
                                    pT[:, col0:512], ps[:, col0:512], Exp,
                                    scale=scale)
                                pTs[t] = pT
                                if ti >= D:
                                    vmms(t_seq[ti - D])
                            # proj tiles read ALL heads' aT of chunk c-1;
                            # at h==0 the previous chunk's last head's
                            # transposes are only emitted at ti==6, so its
                            # proj tiles must come after (reads emitted
                            # before writes get no dependency edge)
                            if pe_list and (h > 0 or ti >= 6) and \
                               (ti + 1) * EPH // (n_tk + 3) > \
                               ti * EPH // (n_tk + 3):
                                proj_tile(pe_list.pop(0), c - 1)
                        for e in pe_list:
                            proj_tile(e, c - 1)

                        # Package this head's tail: the last AV matmuls
                        # (their exps are still draining on ACT) plus the
                        # normalize chain; and, later, the A^T transposes.
                        # Both run inside the NEXT head's t-loop.
                        holder = []

                        def make_tail(vmms=vmms, vmms_pair=vmms_pair,
                                      accs=accs, n_tk=n_tk, D=D, c=c,
                                      holder=holder, t_seq=tuple(t_seq),
                                      pq=pq):
                            def emit():
                                if c > 0:
                                    for p in pq:
                                        vmms_pair(p)
                                else:
                                    for t in t_seq[max(0, n_tk - D):]:
                                        vmms(t)
                                # batched normalize: 4 s-blocks land in one
                                # [128,512] bf16 tile; the fp8 hi/lo split
                                # happens AFTER the transpose (identical
                                # math, and bf16 PE transposes are legal
                                # where fp8 ones need stride-2 outputs)
                                an = npool.tile([128, 512], b16, tag="an")
                                for s in range(4):
                                    rec = spool.tile([128, 1], f32,
                                                     tag="rec")
                                    nc.vector.reciprocal(
                                        rec[:], accs[s][:, 128:129])
                                    nc.vector.tensor_scalar_mul(
                                        an[:, 128 * s:128 * (s + 1)],
                                        accs[s][:, 0:128], rec[:])
                                holder.append(an)
                            return emit

                        def make_tr(holder=holder, h=h, c=c):
                            def emit():
                                pt = psP.tile([128, 512], b16, tag="p8",
                                              name="pt8")
                                an = holder[0]
                                for s in range(4):
                                    nc.tensor.transpose(
                                        pt[:, 128 * s:128 * (s + 1)],
                                        an[:, 128 * s:128 * (s + 1)],
                                        id_sb[:])
                                cs = slice(512 * c, 512 * (c + 1))
                                # post-transpose hi/lo split: hi on ACT,
                                # lo = pt - hi on DVE
                                nc.scalar.copy(out=aTh[:, h, cs],
                                               in_=pt[:])
                                nc.vector.tensor_tensor(
                                    aTl[:, h, cs], pt[:], aTh[:, h, cs],
                                    sub)
                            return emit

                        for _, fn in carry:  # flush any unemitted leftovers
                            fn()
                        # emit points must fit inside the NEXT iteration's
                        # t-loop (n_tk=4 when it is a c==0 head)
                        nxt_c0 = (c == 0 and h < HQL - 1)
                        carry = [(1, make_tail()),
                                 (3 if nxt_c0 else 6, make_tr())]

                for _, fn in carry:
                    fn()
                # drain: projection of the last chunk through the 3-bank ring
                for e in range(NE):
                    proj_tile(e, NCH - 1)

    nc.compile()
    return nc


def _rope_tables(T=T):
    j = np.arange(64, dtype=np.float64)
    inv_freq = 1.0 / (BASE_FREQ ** (2.0 * j / HD))
    t = np.arange(T, dtype=np.float64)
    fr = t[:, None] * inv_freq[None, :]          # [T, 64]
    cos = np.cos(fr) / SW                        # fold 1/SW (fp8 w scaling)
    sin = np.sin(fr) / SW
    cos_tbl = np.concatenate([cos, cos], axis=1).T    # [128, T]
    sin_tbl = sin.T                                   # [64, T]
    return cos_tbl.astype(bf16), sin_tbl.astype(bf16)


def _hilo(a):
    """fp8 e4m3 hi/lo split of a float32 array."""
    h = a.astype(f8e4)
    l = (a - h.astype(np.float32)).astype(f8e4)
    return h, l


def _pack_w(w):
    """[KE, M] -> [128, M//128, KE//128, 128]: w_l[p, m, a, j] = w[128a+p, 128m+j]."""
    KE, M = w.shape
    return np.ascontiguousarray(
        w.reshape(KE // 128, 128, M // 128, 128).transpose(1, 2, 0, 3))


def prep_core_inputs(x, wq, wk, wv, wproj):
    cos_tbl, rsin_tbl = _rope_tables()
    tri = np.where(np.arange(128)[None, :] >= np.arange(128)[:, None],
                   0.0, NEG).astype(np.float32)
    ident = np.eye(128, dtype=bf16)

    # shared fp8 splits (sliced per core below)
    wqh, wql = _hilo(wq * SW)
    wkh, wkl = _hilo(wk * SW)
    wvh, wvl = _hilo(wv * SW)
    wph, wpl = _hilo(wproj * SW)

    xs = []
    for b in range(B):
        xt = np.ascontiguousarray(
            x[b].T.reshape(N_EMBD // 128, 128, T).transpose(1, 0, 2))
        xs.append(_hilo(xt.astype(np.float32)))

    def packp(w, cols):
        # [1024, E] -> [128, NE, HQL, 128]: per e-column-tile slabs
        return np.ascontiguousarray(
            w[cols, :].reshape(HQL, 128, N_EMBD // 128, 128)
            .transpose(1, 2, 0, 3))

    in_maps = []
    for ci in range(N_CORES):
        b, g = divmod(ci, TPG)
        qcols = slice(g * HQL * HD, (g + 1) * HQL * HD)
        kvcols = slice(g * HKVL * HD, (g + 1) * HKVL * HD)
        in_maps.append({
            "xh": xs[b][0], "xl": xs[b][1],
            "wqh": _pack_w(wqh[:, qcols]), "wql": _pack_w(wql[:, qcols]),
            "wkh": _pack_w(wkh[:, kvcols]), "wkl": _pack_w(wkl[:, kvcols]),
            "wvh": _pack_w(wvh[:, kvcols]), "wvl": _pack_w(wvl[:, kvcols]),
            "wph": packp(wph, qcols), "wpl": packp(wpl, qcols),
            "cos": cos_tbl, "rsin": rsin_tbl, "tri": tri, "ident": ident,
        })
    return in_maps


_NC_CACHE = {}


def _get_nc():
    if "nc" not in _NC_CACHE:
        _NC_CACHE["nc"] = build_nc()
    return _NC_CACHE["nc"]


def _get_runner():
    """Cached sharded-jit executor over the 8 cores (no donation, so the
    compiled executable is reusable across calls)."""
    if "runner" in _NC_CACHE:
        return _NC_CACHE["runner"]
    import jax
    from jax.sharding import Mesh, PartitionSpec, NamedSharding
    from jax.experimental.shard_map import shard_map
    from concourse import mybir
    from concourse.bass2jax import (_bass_exec_p, install_neuronx_cc_hook,
                                    partition_id_tensor)

    nc = _get_nc()
    install_neuronx_cc_hook()
    pname = nc.partition_id_tensor.name if nc.partition_id_tensor else None
    in_names, out_names, out_avals, zero_shapes = [], [], [], []
    for alloc in nc.m.functions[0].allocations:
        if not isinstance(alloc, mybir.MemoryLocationSet):
            continue
        name = alloc.memorylocations[0].name
        if alloc.kind == "ExternalInput":
            if name != pname:
                in_names.append(name)
        elif alloc.kind == "ExternalOutput":
            out_names.append(name)
            shape = tuple(alloc.tensor_shape)
            dtype = mybir.dt.np(alloc.dtype)
            out_avals.append(jax.core.ShapedArray(shape, dtype))
            zero_shapes.append((shape, dtype))
    all_names = in_names + out_names + ([pname] if pname else [])

    def _body(*args):
        operands = list(args)
        if pname:
            operands.append(partition_id_tensor())
        return tuple(_bass_exec_p.bind(
            *operands, out_avals=tuple(out_avals), in_names=tuple(all_names),
            out_names=tuple(out_names), lowering_input_output_aliases=(),
            sim_require_finite=True, sim_require_nnan=True, nc=nc))

    devices = jax.devices()[:N_CORES]
    mesh = Mesh(np.asarray(devices), ("core",))
    nin = len(in_names) + len(out_names)
    sharded = jax.jit(
        shard_map(_body, mesh=mesh, in_specs=(PartitionSpec("core"),) * nin,
                  out_specs=(PartitionSpec("core"),) * len(out_names),
                  check_rep=False),
        keep_unused=True)
    sh = NamedSharding(mesh, PartitionSpec("core"))
    zeros = [jax.device_put(
        np.zeros((N_CORES * s[0], *s[1:]), dt), sh)
        for s, dt in zero_shapes]

    def run(in_maps):
        concat = [np.concatenate([m[n] for m in in_maps], axis=0)
                  for n in in_names]
        dev_in = [jax.device_put(a, sh) for a in concat]
        outs = sharded(*dev_in, *zeros)
        jax.block_until_ready(outs)
        return [
            {n: np.asarray(outs[i]).reshape(N_CORES, *out_avals[i].shape)[ci]
             for i, n in enumerate(out_names)}
            for ci in range(N_CORES)]

    _NC_CACHE["runner"] = run
    return run


def kernel(x, wq, wk, wv, wproj):
    in_maps = prep_core_inputs(np.asarray(x, dtype=np.float32),
                               np.asarray(wq, dtype=np.float32),
                               np.asarray(wk, dtype=np.float32),
                               np.asarray(wv, dtype=np.float32),
                               np.asarray(wproj, dtype=np.float32))
    results = _get_runner()(in_maps)
    y = np.empty((B, T, N_EMBD), dtype=np.float32)
    for b in range(B):
        acc = results[b * TPG]["yt"].copy()
        for g in range(1, TPG):
            acc += results[b * TPG + g]["yt"]
        y[b] = acc.T / SW
    return y


if __name__ == "__main__":
    rng = np.random.default_rng(0)
    x = rng.standard_normal((B, T, N_EMBD), dtype=np.float32)
    wq_ = (rng.standard_normal((N_EMBD, N_EMBD), dtype=np.float32) * 0.02)
    wk_ = (rng.standard_normal((N_EMBD, HKV * HD), dtype=np.float32) * 0.02)
    wv_ = (rng.standard_normal((N_EMBD, HKV * HD), dtype=np.float32) * 0.02)
    wp_ = (rng.standard_normal((N_EMBD, N_EMBD), dtype=np.float32) * 0.02)
    y = kernel(x, wq_, wk_, wv_, wp_)
    print("out", y.shape, y.dtype, float(np.abs(y).max()))


# revision 80
# speedup vs baseline: 1.2931x; 1.0132x over previous
"""Trainium2 Bass kernel for causal self-attention (GQA + RoPE).

Problem: B=2, T=2048, n_embd=4096, HQ=32 q-heads, HKV=8 kv-heads, HD=128.
  q = rope(x @ wq), k = rope(x @ wk), v = x @ wv
  y = causal_softmax(q k^T / sqrt(HD)) v @ wproj

Sharding (8 cores): core = (b, g), b in {0,1} batch, g in {0..3} head-group.
Each core handles 8 q-heads / 2 kv-heads of one batch sample:
  - wq/wk/wv column-sharded, wproj row-sharded (tensor parallel over heads)
  - final reduce (sum of 4 partial y per batch) done on host in fp32.

Per-core device program:
  A) projections in COMPENSATED fp8 (e4m3 hi+lo splits of x and w, x64
     weight scaling folded into the rope tables / V eviction): per
     (head, chunk) one PSUM accumulates 48 DoubleRow matmuls
     (3 products x 16 k-tile pairs, 256-deep contraction each), then
     fused RoPE evict (bf16 out).  V^T is PE-transposed into V[tok, dv]
     with a ones column appended.
  B) attention per (head, 512-token q-chunk) in bf16: S^T =
     K-block^T-matmul(Q^T), causal tri mask on diag blocks, ACT exp ->
     P^T (bf16), then out[tq,129] += P^T-block.T @ [V|1] (rowsum rides
     in col 128), normalize, split into fp8 hi+lo, PE-transpose ->
     A^T_hi/A^T_lo [dv, t].
  C) y^T = compensated fp8 DoubleRow over head pairs:
     3 products x 4 head-pairs per e-tile, fp32 eviction, DMA out.
"""
import sys

if "/opt/trn_rl_repo" not in sys.path:
    sys.path.insert(0, "/opt/trn_rl_repo")

import math
import numpy as np
import ml_dtypes

B, T, N_EMBD = 2, 2048, 4096
HQ, HKV = 32, 8
HD = 128
N_CORES = 8
TPG = 4                      # tensor-parallel groups per batch
HQL, HKVL = HQ // TPG, HKV // TPG   # 8 q-heads, 2 kv-heads per core
SCALE = 1.0 / math.sqrt(HD)
BASE_FREQ = 10000.0
NEG = -1e30
SW = 64.0                    # fp8 weight pre-scale

bf16 = ml_dtypes.bfloat16
f8e4 = ml_dtypes.float8_e4m3


def build_nc(T=T, KE=N_EMBD, HQL=HQL, HKVL=HKVL, EOUT=N_EMBD, scale=SCALE):
    """Build the per-core Bass program. All shapes hardcoded at trace time."""
    import concourse.tile as tile
    from concourse import bacc, mybir

    f32 = mybir.dt.float32
    b16 = mybir.dt.bfloat16
    fp8 = mybir.dt.float8e4
    Exp = mybir.ActivationFunctionType.Exp
    Copy = mybir.ActivationFunctionType.Copy
    DR = mybir.MatmulPerfMode.DoubleRow
    mult = mybir.AluOpType.mult
    add = mybir.AluOpType.add
    sub = mybir.AluOpType.subtract

    KT = KE // 128          # contraction tiles for projections
    KP = KT // 2            # DoubleRow k-tile pairs
    NKT = T // 128          # token tiles
    NCH = T // 512          # token chunks
    REP = HQL // HKVL

    nc = bacc.Bacc("TRN2", target_bir_lowering=False)

    xh_d = nc.dram_tensor("xh", [128, KT, T], fp8, kind="ExternalInput")
    xl_d = nc.dram_tensor("xl", [128, KT, T], fp8, kind="ExternalInput")
    wqh_d = nc.dram_tensor("wqh", [128, HQL, KT, 128], fp8, kind="ExternalInput")
    wql_d = nc.dram_tensor("wql", [128, HQL, KT, 128], fp8, kind="ExternalInput")
    wkh_d = nc.dram_tensor("wkh", [128, HKVL, KT, 128], fp8, kind="ExternalInput")
    wkl_d = nc.dram_tensor("wkl", [128, HKVL, KT, 128], fp8, kind="ExternalInput")
    wvh_d = nc.dram_tensor("wvh", [128, HKVL, KT, 128], fp8, kind="ExternalInput")
    wvl_d = nc.dram_tensor("wvl", [128, HKVL, KT, 128], fp8, kind="ExternalInput")
    NE = EOUT // 128
    wph_d = nc.dram_tensor("wph", [128, NE, HQL, 128], fp8, kind="ExternalInput")
    wpl_d = nc.dram_tensor("wpl", [128, NE, HQL, 128], fp8, kind="ExternalInput")
    cos_d = nc.dram_tensor("cos", [128, T], b16, kind="ExternalInput")
    sin_d = nc.dram_tensor("rsin", [64, T], b16, kind="ExternalInput")
    tri_d = nc.dram_tensor("tri", [128, 128], f32, kind="ExternalInput")
    id_d = nc.dram_tensor("ident", [128, 128], b16, kind="ExternalInput")
    yt_d = nc.dram_tensor("yt", [EOUT, T], f32, kind="ExternalOutput")

    with tile.TileContext(nc) as tc:
        with tc.tile_pool(name="glob", bufs=1) as glob:
            cos_sb = glob.tile([128, T], b16)
            sin_sb = glob.tile([64, T], b16)
            tri_sb = glob.tile([128, 128], f32)
            id_sb = glob.tile([128, 128], b16)

            qT = glob.tile([128, HQL, T], b16)       # rope(q)^T per head
            kT = glob.tile([128, HKVL, T], b16)      # rope(k)^T per head
            vON = glob.tile([128, HKVL, NKT, 129], b16)  # [tok, dv | 1]
            nc.vector.memset(vON[:, :, :, 128:129], 1.0)
            # exp bias for the fp8-P path (max exp-arg 11.41; e^5.41 < 240)
            pb_sb = glob.tile([128, 1], f32)
            nc.vector.memset(pb_sb[:], -6.0)

            # ---------------- Phase A: projections -------------------------
            with tc.tile_pool(name="xt", bufs=1) as xtp, \
                 tc.tile_pool(name="wld", bufs=8) as wld, \
                 tc.tile_pool(name="rtmp", bufs=1) as rtmp, \
                 tc.tile_pool(name="vtmp", bufs=1) as vtmp, \
                 tc.tile_pool(name="psA", bufs=7, space="PSUM") as psA, \
                 tc.tile_pool(name="psT", bufs=1, space="PSUM") as psT:

                KH = KT // 2  # weight half-slab depth (16 tiles)

                def load_w(wh_d_, wl_d_, m):
                    """Returns f(prod, kp) -> [128, 2, 128] AP of the k-pair.
                    prod 0 -> hi weights, 1 -> lo weights."""
                    slabs = {}
                    for key, w_d_ in (("h", wh_d_), ("l", wl_d_)):
                        wa = wld.tile([128, KH, 128], fp8, tag="w",
                                      name=f"wa{key}")
                        nc.sync.dma_start(out=wa[:], in_=w_d_[:, m, 0:KH, :])
                        wb = wld.tile([128, KH, 128], fp8, tag="w",
                                      name=f"wb{key}")
                        nc.sync.dma_start(out=wb[:], in_=w_d_[:, m, KH:KT, :])
                        slabs[key] = (wa, wb)

                    def get(key, kp):
                        wa, wb = slabs[key]
                        k0 = 2 * kp
                        if k0 < KH:
                            return wa[:, k0:k0 + 2, :]
                        return wb[:, k0 - KH:k0 - KH + 2, :]
                    return get

                # DMA issue order tracks first use: the m0/m1 hi a-slabs and
                # first x tiles land first (first matmul ~2.5us in); lo
                # a-slabs early because passes 1+2 run interleaved per
                # k-pair (pass 2 reuses the same xh tiles); big rope tables
                # follow the b-slabs (first rope is ~40us in); xl last.
                xh_sb = xtp.tile([128, KT, T], fp8)
                xl_sb = xtp.tile([128, KT, T], fp8)

                def load_slab(w_d_, m, half):
                    w = wld.tile([128, KH, 128], fp8, tag="w")
                    lo, hi = (0, KH) if half == 0 else (KH, KT)
                    nc.sync.dma_start(out=w[:], in_=w_d_[:, m, lo:hi, :])
                    return w

                wa0h = load_slab(wqh_d, 0, 0)
                nc.sync.dma_start(out=xh_sb[:, 0, :], in_=xh_d[:, 0, :])
                nc.sync.dma_start(out=xh_sb[:, 1, :], in_=xh_d[:, 1, :])
                wa1h = load_slab(wqh_d, 1, 0)
                wa0l = load_slab(wql_d, 0, 0)
                wa1l = load_slab(wql_d, 1, 0)
                for a in range(2, KT):
                    nc.sync.dma_start(out=xh_sb[:, a, :], in_=xh_d[:, a, :])
                    if a == 10:  # b-halves needed from kp=8
                        wb0h = load_slab(wqh_d, 0, 1)
                        wb1h = load_slab(wqh_d, 1, 1)
                        wb0l = load_slab(wql_d, 0, 1)
                        wb1l = load_slab(wql_d, 1, 1)
                nc.sync.dma_start(out=cos_sb[:], in_=cos_d[:])
                nc.sync.dma_start(out=sin_sb[:], in_=sin_d[:])
                nc.sync.dma_start(out=tri_sb[:], in_=tri_d[:])
                nc.sync.dma_start(out=id_sb[:], in_=id_d[:])
                for a in range(KT):
                    nc.sync.dma_start(out=xl_sb[:, a, :], in_=xl_d[:, a, :])

                def mk_wfn(wa, wb, wal, wbl):
                    slabs = {"h": (wa, wb), "l": (wal, wbl)}

                    def get(key, kp):
                        a, b = slabs[key]
                        k0 = 2 * kp
                        if k0 < KH:
                            return a[:, k0:k0 + 2, :]
                        return b[:, k0 - KH:k0 - KH + 2, :]
                    return get

                def rope_evict(ps, dst, c):
                    # dst = ps * cos + rot64(ps) * sin  (bf16 out);
                    # rot[0:64] = -ps[64:128], rot[64:128] = ps[0:64]
                    cs = slice(512 * c, 512 * (c + 1))
                    t1 = rtmp.tile([128, 512], f32, tag="t1")
                    nc.vector.scalar_tensor_tensor(
                        t1[0:64, :], ps[64:128, :], -1.0, sin_sb[:, cs],
                        op0=mult, op1=mult)
                    nc.vector.tensor_tensor(t1[64:128, :], ps[0:64, :],
                                            sin_sb[:, cs], mult)
                    t2 = rtmp.tile([128, 512], f32, tag="t2")
                    nc.vector.tensor_tensor(t2[:], ps[:], cos_sb[:, cs], mult)
                    nc.vector.tensor_tensor(dst, t2[:], t1[:], add)

                # (prod, x-operand) sequence for the compensated product:
                #   x_hi@w_hi + x_hi@w_lo + x_lo@w_hi
                def dr_chain(ps, wfn, c, kp_order=None):
                    cs = slice(512 * c, 512 * (c + 1))
                    steps = [("h", xh_sb), ("l", xh_sb), ("h", xl_sb)]
                    n = 0
                    for si, (wkey, xsb) in enumerate(steps):
                        for kp in range(KP):
                            nc.tensor.matmul(
                                ps[:], lhsT=wfn(wkey, kp),
                                rhs=xsb[:, 2 * kp:2 * kp + 2, cs],
                                start=(n == 0), stop=(n == 3 * KP - 1),
                                perf_mode=DR)
                            n += 1

                # Startup ramp: q-heads 0+1 run kp-outer, interleaved, over 8
                # live psums so PE issues 8 matmuls per freshly-landed x tile
                # and tracks the DMA (pass 1 follows xh, pass 3 follows xl).
                # The last pass runs unit-major so early units stop (and
                # their rope evicts start on DVE) while PE finishes the rest.
                units = [(0, c) for c in range(NCH)] + \
                        [(1, c) for c in range(NCH)]
                wfns = {0: mk_wfn(wa0h, wb0h, wa0l, wb0l),
                        1: mk_wfn(wa1h, wb1h, wa1l, wb1l)}
                pss = {u: psA.tile([128, 512], f32, tag="pj",
                                   name=f"pj{u[0]}_{u[1]}")
                       for u in units[:-1]}
                pss[units[-1]] = psT.tile([128, 512], f32, tag="tr",
                                          name="pj8")
                # passes 1+2 interleaved per k-pair: both read the same two
                # xh tiles, so PE does 16 matmuls per 2-tile DMA landing and
                # stays ahead of the x stream.
                for kp in range(KP):
                    for wkey in ("h", "l"):
                        for (m, c) in units:
                            nc.tensor.matmul(
                                pss[(m, c)][:], lhsT=wfns[m](wkey, kp),
                                rhs=xh_sb[:, 2 * kp:2 * kp + 2,
                                          512 * c:512 * (c + 1)],
                                start=(wkey == "h" and kp == 0), stop=False,
                                perf_mode=DR)
                for (m, c) in units:
                    for kp in range(KP):
                        nc.tensor.matmul(
                            pss[(m, c)][:], lhsT=wfns[m]("h", kp),
                            rhs=xl_sb[:, 2 * kp:2 * kp + 2,
                                      512 * c:512 * (c + 1)],
                            start=False, stop=(kp == KP - 1),
                            perf_mode=DR)
                    rope_evict(pss[(m, c)], qT[:, m, 512 * c:512 * (c + 1)], c)

                # remaining projections: q-heads 2-7, k-heads, then V heads.
                # Next head's weight slabs are prefetched before the current
                # head's chunk chains are issued (ring bufs sized to hold
                # current 4 + next 4 slabs).
                heads = [("r", qT, wqh_d, wql_d, m) for m in range(2, HQL)] + \
                        [("r", kT, wkh_d, wkl_d, m) for m in range(HKVL)] + \
                        [("v", None, wvh_d, wvl_d, m) for m in range(HKVL)]
                wcur = load_w(heads[0][2], heads[0][3], heads[0][4])
                for i, (kind, dst, wh_d_, wl_d_, m) in enumerate(heads):
                    wnxt = (load_w(heads[i + 1][2], heads[i + 1][3],
                                   heads[i + 1][4])
                            if i + 1 < len(heads) else None)
                    for c in range(NCH):
                        ps = psA.tile([128, 512], f32, tag="pj")
                        dr_chain(ps, wcur, c)
                        if kind == "r":
                            rope_evict(ps, dst[:, m, 512 * c:512 * (c + 1)],
                                       c)
                        else:
                            # V: v^T psum -> (1/SW) sbuf -> PE transpose
                            vt = vtmp.tile([128, 512], b16, tag="vt")
                            nc.scalar.activation(vt[:], ps[:], Copy,
                                                 scale=1.0 / SW)
                            pt = psT.tile([128, 512], b16, tag="tr")
                            for s in range(4):
                                nc.tensor.transpose(
                                    pt[:, 128 * s:128 * (s + 1)],
                                    vt[:, 128 * s:128 * (s + 1)], id_sb[:])
                            for s in range(4):
                                nc.scalar.copy(
                                    out=vON[:, m, 4 * c + s, 0:128],
                                    in_=pt[:, 128 * s:128 * (s + 1)])
                    wcur = wnxt

            # ---------------- Phases B + C ---------------------------------
            with tc.tile_pool(name="late", bufs=1) as late, \
                 tc.tile_pool(name="ppool", bufs=6) as ppool, \
                 tc.tile_pool(name="p2pool", bufs=2) as p2pool, \
                 tc.tile_pool(name="npool", bufs=8) as npool, \
                 tc.tile_pool(name="spool", bufs=4) as spool, \
                 tc.tile_pool(name="psS", bufs=5, space="PSUM") as psS, \
                 tc.tile_pool(name="psP", bufs=1, space="PSUM") as psP, \
                 tc.tile_pool(name="psacc", bufs=1, space="PSUM") as psacc:

                aTh = late.tile([128, HQL, T], fp8)
                aTl = late.tile([128, HQL, T], fp8)
                # fp8 copy of vON for the DoubleRow-paired AV path (c>=1;
                # converted here where SBUF and DVE both have slack)
                vON8 = late.tile([128, HKVL, NKT, 129], fp8)
                for m in range(HKVL):
                    for hf in range(2):
                        nc.vector.tensor_copy(
                            vON8[:, m, 8 * hf:8 * hf + 8, :],
                            vON[:, m, 8 * hf:8 * hf + 8, :])
                # wproj packed per e-column-tile so DMA lands in consumption
                # order (proj tiles only need their own e slabs, not all of
                # wproj, when the phase-A -> B transition is DMA-tight)
                wph_sb = late.tile([128, NE, HQL, 128], fp8)
                wpl_sb = late.tile([128, NE, HQL, 128], fp8)
                for e in range(NE):
                    nc.sync.dma_start(out=wph_sb[:, e], in_=wph_d[:, e])
                    nc.sync.dma_start(out=wpl_sb[:, e], in_=wpl_d[:, e])

                # Phases B+C software-pipelined: while attention runs for
                # chunk c, the output projection for chunk c-1 is interleaved
                # between heads (4 e-tiles per head) so PE fills ACT-wait
                # gaps and the output DMA spreads across the whole run.
                def proj_tile(e, c, drain=False):
                    cs = slice(512 * c, 512 * (c + 1))
                    ps = psS.tile([128, 512], f32, tag="s", name="psp")
                    n = 0
                    for wsb, asb in ((wph_sb, aTh), (wpl_sb, aTh),
                                     (wph_sb, aTl)):
                        for hp in range(HQL // 2):
                            nc.tensor.matmul(
                                ps[:], lhsT=wsb[:, e, 2 * hp:2 * hp + 2, :],
                                rhs=asb[:, 2 * hp:2 * hp + 2, cs],
                                start=(n == 0), stop=(n == 3 * HQL // 2 - 1),
                                perf_mode=DR)
                            n += 1
                    yt = ppool.tile([128, 512], f32, tag="yt", name="yt")
                    # in-loop evicts stay off ACT (it paces the exp chain);
                    # the drain has no exps so it alternates
                    if drain and e % 2 == 1:
                        nc.scalar.copy(out=yt[:], in_=ps[:])
                    else:
                        nc.vector.tensor_copy(yt[:], ps[:])
                    nc.sync.dma_start(
                        out=yt_d[128 * e:128 * (e + 1), 512 * c:512 * (c + 1)],
                        in_=yt[:])

                NE = EOUT // 128
                EPH = NE // HQL  # proj e-tiles interleaved per head
                # carry: [(emit_ti, fn)] — work of the PREVIOUS head (tail
                # AV matmuls + normalize, then A^T transposes) deferred into
                # the current head's t-loop so PE never waits on it inline.
                carry = []
                for c in range(NCH):
                    for h in range(HQL):
                        v = h // REP
                        # two AV chains share each accumulator bank: only
                        # the even chain issues start=True — its whole-bank
                        # pending-zero mark also zero-fills the odd chain's
                        # region on first write (HW zero-region semantics)
                        acc2 = [psacc.tile([128, 258], f32, tag=f"acc{i}",
                                           name=f"acc{i}")
                                for i in range(2)]
                        accs = [acc2[s // 2][:, 129 * (s % 2):
                                             129 * (s % 2) + 129]
                                for s in range(4)]
                        n_tk = 4 * c + 4
                        pTs = {}
                        pT2 = None
                        if c > 0:
                            # fp8 P for the DoubleRow-paired AV path; the
                            # diag tiles' masked column ranges must be
                            # zeroed by THIS tile generation (inherited
                            # bytes have no cross-generation ordering)
                            pT2 = p2pool.tile([128, NKT, 512], fp8,
                                              tag="p2")
                            for j in range(1, 4):
                                nc.vector.memset(
                                    pT2[:, 4 * c + j, 0:128 * j], 0.0)

                        def vmms(t, accs=accs, pTs=pTs, v=v, c=c):
                            # c==0 single-tile bf16 path
                            j = t - 4 * c
                            for s in range(4):
                                if j > s:
                                    continue
                                nc.tensor.matmul(
                                    accs[s],
                                    lhsT=pTs[t][:, 128 * s:128 * (s + 1)],
                                    rhs=vON[:, v, t, :],
                                    start=(t == 0 and s % 2 == 0),
                                    stop=(t == s and s % 2 == 1))

                        def vmms_pair(p, accs=accs, v=v, c=c, pT2=pT2):
                            # c>=1 DoubleRow path: one instr contracts the
                            # t-pair (2p, 2p+1); masked diag regions in pT2
                            # are zero so they contribute nothing
                            j0 = 2 * p - 4 * c
                            for s in range(4):
                                if j0 > s:  # both tiles fully masked
                                    continue
                                nc.tensor.matmul(
                                    accs[s],
                                    lhsT=pT2[:, 2 * p:2 * p + 2,
                                             128 * s:128 * (s + 1)],
                                    rhs=vON8[:, v, 2 * p:2 * p + 2, :],
                                    start=(p == 0 and s % 2 == 0),
                                    stop=((s == 1 and p == 2 * c - 1) or
                                          (s == 3 and p == 2 * c + 1)),
                                    perf_mode=DR)

                        # proj tiles of the previous chunk, interleaved into
                        # the t-loop (own psum bank) to fill ACT-paced gaps.
                        # h==0 gets none: its proj tiles would race the
                        # previous chunk's last transposes, so its share is
                        # redistributed over heads 1..7 (race-safe at any ti)
                        pe_list = ([] if c == 0 or h == 0 else
                                   list(range((h - 1) * NE // (HQL - 1),
                                              h * NE // (HQL - 1))))
                        D = 3
                        # diag tiles spread through the loop (one after
                        # every c full tiles): their ACT exp work and DVE
                        # tri-adds never bunch at the tail, and the loop
                        # ends on the smallest (128-col) exp.
                        t_seq = []
                        for i in range(4):
                            t_seq += list(range(c * i, c * (i + 1)))
                            t_seq.append(4 * c + i)
                        # pair p becomes ready at the position of its later
                        # tile; emit its AV matmuls D positions later
                        ready = {}
                        for ti, t in enumerate(t_seq):
                            ready[t // 2] = ti
                        pair_seq = sorted(ready, key=lambda p: ready[p])
                        pq = list(pair_seq)
                        npe0 = len(pe_list)
                        for ti, t in enumerate(t_seq):
                            j = t - 4 * c  # >= 0 on diagonal-group tiles
                            col0 = 128 * j if j > 0 else 0
                            ps = psS.tile([128, 512], f32, tag="s")
                            nc.tensor.matmul(
                                ps[:, col0:512],
                                lhsT=kT[:, v, 128 * t:128 * (t + 1)],
                                rhs=qT[:, h, 512 * c + col0:512 * (c + 1)],
                                start=True, stop=True)
                            while carry and carry[0][0] <= ti:
                                carry.pop(0)[1]()
                            if j >= 0:
                                nc.vector.tensor_tensor(
                                    ps[:, 128 * j:128 * (j + 1)],
                                    ps[:, 128 * j:128 * (j + 1)],
                                    tri_sb[:], add)
                            if c > 0:
                                nc.scalar.activation(
                                    pT2[:, t, col0:512], ps[:, col0:512],
                                    Exp, scale=scale, bias=pb_sb[:])
                                while pq and ready[pq[0]] <= ti - D:
                                    vmms_pair(pq.pop(0))
                            else:
                                pT = ppool.tile([128, 512], b16, tag="pT")
                                nc.scalar.activation(
                                    pT[:, col0:512], ps[:, col0:512], Exp,
                                    scale=scale)
                                pTs[t] = pT
                                if ti >= D:
                                    vmms(t_seq[ti - D])
                            if pe_list and \
                               (ti + 1) * npe0 // (n_tk + 2) > \
                               ti * npe0 // (n_tk + 2):
                                proj_tile(pe_list.pop(0), c - 1)
                        for e in pe_list:
                            proj_tile(e, c - 1)

                        # Package this head's tail: the last AV matmuls
                        # (their exps are still draining on ACT) plus the
                        # normalize chain; and, later, the A^T transposes.
                        # Both run inside the NEXT head's t-loop.
                        holder = []

                        def make_tail(vmms=vmms, vmms_pair=vmms_pair,
                                      accs=accs, n_tk=n_tk, D=D, c=c,
                                      holder=holder, t_seq=tuple(t_seq),
                                      pq=pq):
                            def emit():
                                if c > 0:
                                    for p in pq:
                                        vmms_pair(p)
                                else:
                                    for t in t_seq[max(0, n_tk - D):]:
                                        vmms(t)
                                # batched normalize: 4 s-blocks land in one
                                # [128,512] bf16 tile; the fp8 hi/lo split
                                # happens AFTER the transpose (identical
                                # math, and bf16 PE transposes are legal
                                # where fp8 ones need stride-2 outputs)
                                an = npool.tile([128, 512], b16, tag="an")
                                for s in range(4):
                                    rec = spool.tile([128, 1], f32,
                                                     tag="rec")
                                    nc.vector.reciprocal(
                                        rec[:], accs[s][:, 128:129])
                                    nc.vector.tensor_scalar_mul(
                                        an[:, 128 * s:128 * (s + 1)],
                                        accs[s][:, 0:128], rec[:])
                                holder.append(an)
                            return emit

                        def make_tr(holder=holder, h=h, c=c):
                            def emit():
                                pt = psP.tile([128, 512], b16, tag="p8",
                                              name="pt8")
                                an = holder[0]
                                for s in range(4):
                                    nc.tensor.transpose(
                                        pt[:, 128 * s:128 * (s + 1)],
                                        an[:, 128 * s:128 * (s + 1)],
                                        id_sb[:])
                                cs = slice(512 * c, 512 * (c + 1))
                                # post-transpose hi/lo split: hi on ACT,
                                # lo = pt - hi on DVE
                                nc.scalar.copy(out=aTh[:, h, cs],
                                               in_=pt[:])
                                nc.vector.tensor_tensor(
                                    aTl[:, h, cs], pt[:], aTh[:, h, cs],
                                    sub)
                            return emit

                        for _, fn in carry:  # flush any unemitted leftovers
                            fn()
                        # emit points must fit inside the NEXT iteration's
                        # t-loop (n_tk=4 when it is a c==0 head)
                        nxt_c0 = (c == 0 and h < HQL - 1)
                        carry = [(1, make_tail()),
                                 (3 if nxt_c0 else 6, make_tr())]

                for _, fn in carry:
                    fn()
                # drain: projection of the last chunk through the 3-bank ring
                for e in range(NE):
                    proj_tile(e, NCH - 1)

    nc.compile()
    return nc


def _rope_tables(T=T):
    j = np.arange(64, dtype=np.float64)
    inv_freq = 1.0 / (BASE_FREQ ** (2.0 * j / HD))
    t = np.arange(T, dtype=np.float64)
    fr = t[:, None] * inv_freq[None, :]          # [T, 64]
    cos = np.cos(fr) / SW                        # fold 1/SW (fp8 w scaling)
    sin = np.sin(fr) / SW
    cos_tbl = np.concatenate([cos, cos], axis=1).T    # [128, T]
    sin_tbl = sin.T                                   # [64, T]
    return cos_tbl.astype(bf16), sin_tbl.astype(bf16)


def _hilo(a):
    """fp8 e4m3 hi/lo split of a float32 array."""
    h = a.astype(f8e4)
    l = (a - h.astype(np.float32)).astype(f8e4)
    return h, l


def _pack_w(w):
    """[KE, M] -> [128, M//128, KE//128, 128]: w_l[p, m, a, j] = w[128a+p, 128m+j]."""
    KE, M = w.shape
    return np.ascontiguousarray(
        w.reshape(KE // 128, 128, M // 128, 128).transpose(1, 2, 0, 3))


def prep_core_inputs(x, wq, wk, wv, wproj):
    cos_tbl, rsin_tbl = _rope_tables()
    tri = np.where(np.arange(128)[None, :] >= np.arange(128)[:, None],
                   0.0, NEG).astype(np.float32)
    ident = np.eye(128, dtype=bf16)

    # shared fp8 splits (sliced per core below)
    wqh, wql = _hilo(wq * SW)
    wkh, wkl = _hilo(wk * SW)
    wvh, wvl = _hilo(wv * SW)
    wph, wpl = _hilo(wproj * SW)

    xs = []
    for b in range(B):
        xt = np.ascontiguousarray(
            x[b].T.reshape(N_EMBD // 128, 128, T).transpose(1, 0, 2))
        xs.append(_hilo(xt.astype(np.float32)))

    def packp(w, cols):
        # [1024, E] -> [128, NE, HQL, 128]: per e-column-tile slabs
        return np.ascontiguousarray(
            w[cols, :].reshape(HQL, 128, N_EMBD // 128, 128)
            .transpose(1, 2, 0, 3))

    in_maps = []
    for ci in range(N_CORES):
        b, g = divmod(ci, TPG)
        qcols = slice(g * HQL * HD, (g + 1) * HQL * HD)
        kvcols = slice(g * HKVL * HD, (g + 1) * HKVL * HD)
        in_maps.append({
            "xh": xs[b][0], "xl": xs[b][1],
            "wqh": _pack_w(wqh[:, qcols]), "wql": _pack_w(wql[:, qcols]),
            "wkh": _pack_w(wkh[:, kvcols]), "wkl": _pack_w(wkl[:, kvcols]),
            "wvh": _pack_w(wvh[:, kvcols]), "wvl": _pack_w(wvl[:, kvcols]),
            "wph": packp(wph, qcols), "wpl": packp(wpl, qcols),
            "cos": cos_tbl, "rsin": rsin_tbl, "tri": tri, "ident": ident,
        })
    return in_maps


_NC_CACHE = {}


def _get_nc():
    if "nc" not in _NC_CACHE:
        _NC_CACHE["nc"] = build_nc()
    return _NC_CACHE["nc"]


def _get_runner():
    """Cached sharded-jit executor over the 8 cores (no donation, so the
    compiled executable is reusable across calls)."""
    if "runner" in _NC_CACHE:
        return _NC_CACHE["runner"]
    import jax
    from jax.sharding import Mesh, PartitionSpec, NamedSharding
    from jax.experimental.shard_map import shard_map
    from concourse import mybir
    from concourse.bass2jax import (_bass_exec_p, install_neuronx_cc_hook,
                                    partition_id_tensor)

    nc = _get_nc()
    install_neuronx_cc_hook()
    pname = nc.partition_id_tensor.name if nc.partition_id_tensor else None
    in_names, out_names, out_avals, zero_shapes = [], [], [], []
    for alloc in nc.m.functions[0].allocations:
        if not isinstance(alloc, mybir.MemoryLocationSet):
            continue
        name = alloc.memorylocations[0].name
        if alloc.kind == "ExternalInput":
            if name != pname:
                in_names.append(name)
        elif alloc.kind == "ExternalOutput":
            out_names.append(name)
            shape = tuple(alloc.tensor_shape)
            dtype = mybir.dt.np(alloc.dtype)
            out_avals.append(jax.core.ShapedArray(shape, dtype))
            zero_shapes.append((shape, dtype))
    all_names = in_names + out_names + ([pname] if pname else [])

    def _body(*args):
        operands = list(args)
        if pname:
            operands.append(partition_id_tensor())
        return tuple(_bass_exec_p.bind(
            *operands, out_avals=tuple(out_avals), in_names=tuple(all_names),
            out_names=tuple(out_names), lowering_input_output_aliases=(),
            sim_require_finite=True, sim_require_nnan=True, nc=nc))

    devices = jax.devices()[:N_CORES]
    mesh = Mesh(np.asarray(devices), ("core",))
    nin = len(in_names) + len(out_names)
    sharded = jax.jit(
        shard_map(_body, mesh=mesh, in_specs=(PartitionSpec("core"),) * nin,
                  out_specs=(PartitionSpec("core"),) * len(out_names),
                  check_rep=False),
        keep_unused=True)
    sh = NamedSharding(mesh, PartitionSpec("core"))
    zeros = [jax.device_put(
        np.zeros((N_CORES * s[0], *s[1:]), dt), sh)
        for s, dt in zero_shapes]

    def run(in_maps):
        concat = [np.concatenate([m[n] for m in in_maps], axis=0)
                  for n in in_names]
        dev_in = [jax.device_put(a, sh) for a in concat]
        outs = sharded(*dev_in, *zeros)
        jax.block_until_ready(outs)
        return [
            {n: np.asarray(outs[i]).reshape(N_CORES, *out_avals[i].shape)[ci]
             for i, n in enumerate(out_names)}
            for ci in range(N_CORES)]

    _NC_CACHE["runner"] = run
    return run


def kernel(x, wq, wk, wv, wproj):
    in_maps = prep_core_inputs(np.asarray(x, dtype=np.float32),
                               np.asarray(wq, dtype=np.float32),
                               np.asarray(wk, dtype=np.float32),
                               np.asarray(wv, dtype=np.float32),
                               np.asarray(wproj, dtype=np.float32))
    results = _get_runner()(in_maps)
    y = np.empty((B, T, N_EMBD), dtype=np.float32)
    for b in range(B):
        acc = results[b * TPG]["yt"].copy()
        for g in range(1, TPG):
            acc += results[b * TPG + g]["yt"]
        y[b] = acc.T / SW
    return y


if __name__ == "__main__":
    rng = np.random.default_rng(0)
    x = rng.standard_normal((B, T, N_EMBD), dtype=np.float32)
    wq_ = (rng.standard_normal((N_EMBD, N_EMBD), dtype=np.float32) * 0.02)
    wk_ = (rng.standard_normal((N_EMBD, HKV * HD), dtype=np.float32) * 0.02)
    wv_ = (rng.standard_normal((N_EMBD, HKV * HD), dtype=np.float32) * 0.02)
    wp_ = (rng.standard_normal((N_EMBD, N_EMBD), dtype=np.float32) * 0.02)
    y = kernel(x, wq_, wk_, wv_, wp_)
    print("out", y.shape, y.dtype, float(np.abs(y).max()))


# revision 85
# speedup vs baseline: 1.2983x; 1.0040x over previous
"""Trainium2 Bass kernel for causal self-attention (GQA + RoPE).

Problem: B=2, T=2048, n_embd=4096, HQ=32 q-heads, HKV=8 kv-heads, HD=128.
  q = rope(x @ wq), k = rope(x @ wk), v = x @ wv
  y = causal_softmax(q k^T / sqrt(HD)) v @ wproj

Sharding (8 cores): core = (b, g), b in {0,1} batch, g in {0..3} head-group.
Each core handles 8 q-heads / 2 kv-heads of one batch sample:
  - wq/wk/wv column-sharded, wproj row-sharded (tensor parallel over heads)
  - final reduce (sum of 4 partial y per batch) done on host in fp32.

Per-core device program:
  A) projections in COMPENSATED fp8 (e4m3 hi+lo splits of x and w, x64
     weight scaling folded into the rope tables / V eviction): per
     (head, chunk) one PSUM accumulates 48 DoubleRow matmuls
     (3 products x 16 k-tile pairs, 256-deep contraction each), then
     fused RoPE evict (bf16 out).  V^T is PE-transposed into V[tok, dv]
     with a ones column appended.
  B) attention per (head, 512-token q-chunk) in bf16: S^T =
     K-block^T-matmul(Q^T), causal tri mask on diag blocks, ACT exp ->
     P^T (bf16), then out[tq,129] += P^T-block.T @ [V|1] (rowsum rides
     in col 128), normalize, split into fp8 hi+lo, PE-transpose ->
     A^T_hi/A^T_lo [dv, t].
  C) y^T = compensated fp8 DoubleRow over head pairs:
     3 products x 4 head-pairs per e-tile, fp32 eviction, DMA out.
"""
import sys

if "/opt/trn_rl_repo" not in sys.path:
    sys.path.insert(0, "/opt/trn_rl_repo")

import math
import numpy as np
import ml_dtypes

B, T, N_EMBD = 2, 2048, 4096
HQ, HKV = 32, 8
HD = 128
N_CORES = 8
TPG = 4                      # tensor-parallel groups per batch
HQL, HKVL = HQ // TPG, HKV // TPG   # 8 q-heads, 2 kv-heads per core
SCALE = 1.0 / math.sqrt(HD)
BASE_FREQ = 10000.0
NEG = -1e30
SW = 64.0                    # fp8 weight pre-scale

bf16 = ml_dtypes.bfloat16
f8e4 = ml_dtypes.float8_e4m3


def build_nc(T=T, KE=N_EMBD, HQL=HQL, HKVL=HKVL, EOUT=N_EMBD, scale=SCALE):
    """Build the per-core Bass program. All shapes hardcoded at trace time."""
    import concourse.tile as tile
    from concourse import bacc, mybir

    f32 = mybir.dt.float32
    b16 = mybir.dt.bfloat16
    fp8 = mybir.dt.float8e4
    Exp = mybir.ActivationFunctionType.Exp
    Copy = mybir.ActivationFunctionType.Copy
    DR = mybir.MatmulPerfMode.DoubleRow
    mult = mybir.AluOpType.mult
    add = mybir.AluOpType.add
    sub = mybir.AluOpType.subtract

    KT = KE // 128          # contraction tiles for projections
    KP = KT // 2            # DoubleRow k-tile pairs
    NKT = T // 128          # token tiles
    NCH = T // 512          # token chunks
    REP = HQL // HKVL

    nc = bacc.Bacc("TRN2", target_bir_lowering=False)

    xh_d = nc.dram_tensor("xh", [128, KT, T], fp8, kind="ExternalInput")
    xl_d = nc.dram_tensor("xl", [128, KT, T], fp8, kind="ExternalInput")
    wqh_d = nc.dram_tensor("wqh", [128, HQL, KT, 128], fp8, kind="ExternalInput")
    wql_d = nc.dram_tensor("wql", [128, HQL, KT, 128], fp8, kind="ExternalInput")
    wkh_d = nc.dram_tensor("wkh", [128, HKVL, KT, 128], fp8, kind="ExternalInput")
    wkl_d = nc.dram_tensor("wkl", [128, HKVL, KT, 128], fp8, kind="ExternalInput")
    wvh_d = nc.dram_tensor("wvh", [128, HKVL, KT, 128], fp8, kind="ExternalInput")
    wvl_d = nc.dram_tensor("wvl", [128, HKVL, KT, 128], fp8, kind="ExternalInput")
    NE = EOUT // 128
    wph_d = nc.dram_tensor("wph", [128, NE, HQL, 128], fp8, kind="ExternalInput")
    wpl_d = nc.dram_tensor("wpl", [128, NE, HQL, 128], fp8, kind="ExternalInput")
    cos_d = nc.dram_tensor("cos", [128, T], b16, kind="ExternalInput")
    sin_d = nc.dram_tensor("rsin", [64, T], b16, kind="ExternalInput")
    tri_d = nc.dram_tensor("tri", [128, 128], f32, kind="ExternalInput")
    id_d = nc.dram_tensor("ident", [128, 128], b16, kind="ExternalInput")
    yt_d = nc.dram_tensor("yt", [EOUT, T], f32, kind="ExternalOutput")

    with tile.TileContext(nc) as tc:
        with tc.tile_pool(name="glob", bufs=1) as glob:
            cos_sb = glob.tile([128, T], b16)
            sin_sb = glob.tile([64, T], b16)
            tri_sb = glob.tile([128, 128], f32)
            id_sb = glob.tile([128, 128], b16)

            qT = glob.tile([128, HQL, T], b16)       # rope(q)^T per head
            kT = glob.tile([128, HKVL, T], b16)      # rope(k)^T per head
            vON = glob.tile([128, HKVL, NKT, 129], b16)  # [tok, dv | 1]
            nc.vector.memset(vON[:, :, :, 128:129], 1.0)
            # exp bias for the fp8-P path (max exp-arg 11.41; e^5.41 < 240)
            pb_sb = glob.tile([128, 1], f32)
            nc.vector.memset(pb_sb[:], -6.0)

            # ---------------- Phase A: projections -------------------------
            with tc.tile_pool(name="xt", bufs=1) as xtp, \
                 tc.tile_pool(name="wld", bufs=8) as wld, \
                 tc.tile_pool(name="rtmp", bufs=1) as rtmp, \
                 tc.tile_pool(name="vtmp", bufs=1) as vtmp, \
                 tc.tile_pool(name="psA", bufs=7, space="PSUM") as psA, \
                 tc.tile_pool(name="psT", bufs=1, space="PSUM") as psT:

                KH = KT // 2  # weight half-slab depth (16 tiles)

                def load_w(wh_d_, wl_d_, m):
                    """Returns f(prod, kp) -> [128, 2, 128] AP of the k-pair.
                    prod 0 -> hi weights, 1 -> lo weights."""
                    slabs = {}
                    for key, w_d_ in (("h", wh_d_), ("l", wl_d_)):
                        wa = wld.tile([128, KH, 128], fp8, tag="w",
                                      name=f"wa{key}")
                        nc.sync.dma_start(out=wa[:], in_=w_d_[:, m, 0:KH, :])
                        wb = wld.tile([128, KH, 128], fp8, tag="w",
                                      name=f"wb{key}")
                        nc.sync.dma_start(out=wb[:], in_=w_d_[:, m, KH:KT, :])
                        slabs[key] = (wa, wb)

                    def get(key, kp):
                        wa, wb = slabs[key]
                        k0 = 2 * kp
                        if k0 < KH:
                            return wa[:, k0:k0 + 2, :]
                        return wb[:, k0 - KH:k0 - KH + 2, :]
                    return get

                # DMA issue order tracks first use: the m0/m1 hi a-slabs and
                # first x tiles land first (first matmul ~2.5us in); lo
                # a-slabs early because passes 1+2 run interleaved per
                # k-pair (pass 2 reuses the same xh tiles); big rope tables
                # follow the b-slabs (first rope is ~40us in); xl last.
                xh_sb = xtp.tile([128, KT, T], fp8)
                xl_sb = xtp.tile([128, KT, T], fp8)

                def load_slab(w_d_, m, half):
                    w = wld.tile([128, KH, 128], fp8, tag="w")
                    lo, hi = (0, KH) if half == 0 else (KH, KT)
                    nc.sync.dma_start(out=w[:], in_=w_d_[:, m, lo:hi, :])
                    return w

                wa0h = load_slab(wqh_d, 0, 0)
                nc.sync.dma_start(out=xh_sb[:, 0, :], in_=xh_d[:, 0, :])
                nc.sync.dma_start(out=xh_sb[:, 1, :], in_=xh_d[:, 1, :])
                wa1h = load_slab(wqh_d, 1, 0)
                wa0l = load_slab(wql_d, 0, 0)
                wa1l = load_slab(wql_d, 1, 0)
                for a in range(2, KT):
                    nc.sync.dma_start(out=xh_sb[:, a, :], in_=xh_d[:, a, :])
                    if a == 10:  # b-halves needed from kp=8
                        wb0h = load_slab(wqh_d, 0, 1)
                        wb1h = load_slab(wqh_d, 1, 1)
                        wb0l = load_slab(wql_d, 0, 1)
                        wb1l = load_slab(wql_d, 1, 1)
                nc.sync.dma_start(out=cos_sb[:], in_=cos_d[:])
                nc.sync.dma_start(out=sin_sb[:], in_=sin_d[:])
                nc.sync.dma_start(out=tri_sb[:], in_=tri_d[:])
                nc.sync.dma_start(out=id_sb[:], in_=id_d[:])
                for a in range(KT):
                    nc.sync.dma_start(out=xl_sb[:, a, :], in_=xl_d[:, a, :])

                def mk_wfn(wa, wb, wal, wbl):
                    slabs = {"h": (wa, wb), "l": (wal, wbl)}

                    def get(key, kp):
                        a, b = slabs[key]
                        k0 = 2 * kp
                        if k0 < KH:
                            return a[:, k0:k0 + 2, :]
                        return b[:, k0 - KH:k0 - KH + 2, :]
                    return get

                def rope_evict(ps, dst, c):
                    # dst = ps * cos + rot64(ps) * sin  (bf16 out);
                    # rot[0:64] = -ps[64:128], rot[64:128] = ps[0:64]
                    cs = slice(512 * c, 512 * (c + 1))
                    t1 = rtmp.tile([128, 512], f32, tag="t1")
                    nc.vector.scalar_tensor_tensor(
                        t1[0:64, :], ps[64:128, :], -1.0, sin_sb[:, cs],
                        op0=mult, op1=mult)
                    nc.vector.tensor_tensor(t1[64:128, :], ps[0:64, :],
                                            sin_sb[:, cs], mult)
                    t2 = rtmp.tile([128, 512], f32, tag="t2")
                    nc.vector.tensor_tensor(t2[:], ps[:], cos_sb[:, cs], mult)
                    nc.vector.tensor_tensor(dst, t2[:], t1[:], add)

                # (prod, x-operand) sequence for the compensated product:
                #   x_hi@w_hi + x_hi@w_lo + x_lo@w_hi
                def dr_chain(ps, wfn, c, kp_order=None):
                    cs = slice(512 * c, 512 * (c + 1))
                    steps = [("h", xh_sb), ("l", xh_sb), ("h", xl_sb)]
                    n = 0
                    for si, (wkey, xsb) in enumerate(steps):
                        for kp in range(KP):
                            nc.tensor.matmul(
                                ps[:], lhsT=wfn(wkey, kp),
                                rhs=xsb[:, 2 * kp:2 * kp + 2, cs],
                                start=(n == 0), stop=(n == 3 * KP - 1),
                                perf_mode=DR)
                            n += 1

                # Startup ramp: q-heads 0+1 run kp-outer, interleaved, over 8
                # live psums so PE issues 8 matmuls per freshly-landed x tile
                # and tracks the DMA (pass 1 follows xh, pass 3 follows xl).
                # The last pass runs unit-major so early units stop (and
                # their rope evicts start on DVE) while PE finishes the rest.
                units = [(0, c) for c in range(NCH)] + \
                        [(1, c) for c in range(NCH)]
                wfns = {0: mk_wfn(wa0h, wb0h, wa0l, wb0l),
                        1: mk_wfn(wa1h, wb1h, wa1l, wb1l)}
                pss = {u: psA.tile([128, 512], f32, tag="pj",
                                   name=f"pj{u[0]}_{u[1]}")
                       for u in units[:-1]}
                pss[units[-1]] = psT.tile([128, 512], f32, tag="tr",
                                          name="pj8")
                # passes 1+2 interleaved per k-pair: both read the same two
                # xh tiles, so PE does 16 matmuls per 2-tile DMA landing and
                # stays ahead of the x stream.
                for kp in range(KP):
                    for wkey in ("h", "l"):
                        for (m, c) in units:
                            nc.tensor.matmul(
                                pss[(m, c)][:], lhsT=wfns[m](wkey, kp),
                                rhs=xh_sb[:, 2 * kp:2 * kp + 2,
                                          512 * c:512 * (c + 1)],
                                start=(wkey == "h" and kp == 0), stop=False,
                                perf_mode=DR)
                for (m, c) in units:
                    for kp in range(KP):
                        nc.tensor.matmul(
                            pss[(m, c)][:], lhsT=wfns[m]("h", kp),
                            rhs=xl_sb[:, 2 * kp:2 * kp + 2,
                                      512 * c:512 * (c + 1)],
                            start=False, stop=(kp == KP - 1),
                            perf_mode=DR)
                    rope_evict(pss[(m, c)], qT[:, m, 512 * c:512 * (c + 1)], c)

                # remaining projections: q-heads 2-7, k-heads, then V heads.
                # Next head's weight slabs are prefetched before the current
                # head's chunk chains are issued (ring bufs sized to hold
                # current 4 + next 4 slabs).
                heads = [("r", qT, wqh_d, wql_d, m) for m in range(2, HQL)] + \
                        [("r", kT, wkh_d, wkl_d, m) for m in range(HKVL)] + \
                        [("v", None, wvh_d, wvl_d, m) for m in range(HKVL)]
                wcur = load_w(heads[0][2], heads[0][3], heads[0][4])
                for i, (kind, dst, wh_d_, wl_d_, m) in enumerate(heads):
                    wnxt = (load_w(heads[i + 1][2], heads[i + 1][3],
                                   heads[i + 1][4])
                            if i + 1 < len(heads) else None)
                    for c in range(NCH):
                        ps = psA.tile([128, 512], f32, tag="pj")
                        dr_chain(ps, wcur, c)
                        if kind == "r":
                            rope_evict(ps, dst[:, m, 512 * c:512 * (c + 1)],
                                       c)
                        else:
                            # V: v^T psum -> (1/SW) sbuf -> PE transpose
                            vt = vtmp.tile([128, 512], b16, tag="vt")
                            nc.scalar.activation(vt[:], ps[:], Copy,
                                                 scale=1.0 / SW)
                            pt = psT.tile([128, 512], b16, tag="tr")
                            for s in range(4):
                                nc.tensor.transpose(
                                    pt[:, 128 * s:128 * (s + 1)],
                                    vt[:, 128 * s:128 * (s + 1)], id_sb[:])
                            for s in range(4):
                                nc.scalar.copy(
                                    out=vON[:, m, 4 * c + s, 0:128],
                                    in_=pt[:, 128 * s:128 * (s + 1)])
                    wcur = wnxt

            # ---------------- Phases B + C ---------------------------------
            with tc.tile_pool(name="late", bufs=1) as late, \
                 tc.tile_pool(name="ppool", bufs=7) as ppool, \
                 tc.tile_pool(name="p2pool", bufs=2) as p2pool, \
                 tc.tile_pool(name="npool", bufs=8) as npool, \
                 tc.tile_pool(name="spool", bufs=4) as spool, \
                 tc.tile_pool(name="psS", bufs=5, space="PSUM") as psS, \
                 tc.tile_pool(name="psP", bufs=1, space="PSUM") as psP, \
                 tc.tile_pool(name="psacc", bufs=1, space="PSUM") as psacc:

                aTh = late.tile([128, HQL, T], fp8)
                aTl = late.tile([128, HQL, T], fp8)
                # fp8 copy of vON for the DoubleRow-paired AV path (c>=1;
                # converted here where SBUF and DVE both have slack)
                vON8 = late.tile([128, HKVL, NKT, 129], fp8)
                for m in range(HKVL):
                    for hf in range(2):
                        nc.vector.tensor_copy(
                            vON8[:, m, 8 * hf:8 * hf + 8, :],
                            vON[:, m, 8 * hf:8 * hf + 8, :])
                # wproj packed per e-column-tile so DMA lands in consumption
                # order (proj tiles only need their own e slabs, not all of
                # wproj, when the phase-A -> B transition is DMA-tight)
                wph_sb = late.tile([128, NE, HQL, 128], fp8)
                wpl_sb = late.tile([128, NE, HQL, 128], fp8)
                for e in range(NE):
                    nc.sync.dma_start(out=wph_sb[:, e], in_=wph_d[:, e])
                    nc.sync.dma_start(out=wpl_sb[:, e], in_=wpl_d[:, e])

                # Phases B+C software-pipelined: while attention runs for
                # chunk c, the output projection for chunk c-1 is interleaved
                # between heads (4 e-tiles per head) so PE fills ACT-wait
                # gaps and the output DMA spreads across the whole run.
                def proj_tile(e, c, drain=False):
                    cs = slice(512 * c, 512 * (c + 1))
                    ps = psS.tile([128, 512], f32, tag="s", name="psp")
                    n = 0
                    for wsb, asb in ((wph_sb, aTh), (wpl_sb, aTh),
                                     (wph_sb, aTl)):
                        for hp in range(HQL // 2):
                            nc.tensor.matmul(
                                ps[:], lhsT=wsb[:, e, 2 * hp:2 * hp + 2, :],
                                rhs=asb[:, 2 * hp:2 * hp + 2, cs],
                                start=(n == 0), stop=(n == 3 * HQL // 2 - 1),
                                perf_mode=DR)
                            n += 1
                    yt = ppool.tile([128, 512], f32, tag="yt", name="yt")
                    # in-loop evicts stay off ACT (it paces the exp chain);
                    # the drain has no exps so it alternates
                    if drain and e % 2 == 1:
                        nc.scalar.copy(out=yt[:], in_=ps[:])
                    else:
                        nc.vector.tensor_copy(yt[:], ps[:])
                    nc.sync.dma_start(
                        out=yt_d[128 * e:128 * (e + 1), 512 * c:512 * (c + 1)],
                        in_=yt[:])

                NE = EOUT // 128
                EPH = NE // HQL  # proj e-tiles interleaved per head
                # carry: [(emit_ti, fn)] — work of the PREVIOUS head (tail
                # AV matmuls + normalize, then A^T transposes) deferred into
                # the current head's t-loop so PE never waits on it inline.
                carry = []
                for c in range(NCH):
                    for h in range(HQL):
                        v = h // REP
                        # two AV chains share each accumulator bank: only
                        # the even chain issues start=True — its whole-bank
                        # pending-zero mark also zero-fills the odd chain's
                        # region on first write (HW zero-region semantics)
                        acc2 = [psacc.tile([128, 258], f32, tag=f"acc{i}",
                                           name=f"acc{i}")
                                for i in range(2)]
                        accs = [acc2[s // 2][:, 129 * (s % 2):
                                             129 * (s % 2) + 129]
                                for s in range(4)]
                        n_tk = 4 * c + 4
                        pTs = {}
                        pT2 = None
                        if c > 0:
                            # fp8 P for the DoubleRow-paired AV path; the
                            # diag tiles' masked column ranges must be
                            # zeroed by THIS tile generation (inherited
                            # bytes have no cross-generation ordering)
                            pT2 = p2pool.tile([128, NKT, 512], fp8,
                                              tag="p2")
                            for j in range(1, 4):
                                nc.vector.memset(
                                    pT2[:, 4 * c + j, 0:128 * j], 0.0)

                        def vmms(t, accs=accs, pTs=pTs, v=v, c=c):
                            # c==0 single-tile bf16 path
                            j = t - 4 * c
                            for s in range(4):
                                if j > s:
                                    continue
                                nc.tensor.matmul(
                                    accs[s],
                                    lhsT=pTs[t][:, 128 * s:128 * (s + 1)],
                                    rhs=vON[:, v, t, :],
                                    start=(t == 0 and s % 2 == 0),
                                    stop=(t == s and s % 2 == 1))

                        def vmms_pair(p, accs=accs, v=v, c=c, pT2=pT2):
                            # c>=1 DoubleRow path: one instr contracts the
                            # t-pair (2p, 2p+1); masked diag regions in pT2
                            # are zero so they contribute nothing
                            j0 = 2 * p - 4 * c
                            for s in range(4):
                                if j0 > s:  # both tiles fully masked
                                    continue
                                nc.tensor.matmul(
                                    accs[s],
                                    lhsT=pT2[:, 2 * p:2 * p + 2,
                                             128 * s:128 * (s + 1)],
                                    rhs=vON8[:, v, 2 * p:2 * p + 2, :],
                                    start=(p == 0 and s % 2 == 0),
                                    stop=((s == 1 and p == 2 * c - 1) or
                                          (s == 3 and p == 2 * c + 1)),
                                    perf_mode=DR)

                        # proj tiles of the previous chunk, interleaved into
                        # the t-loop (own psum bank) to fill ACT-paced gaps.
                        # h==0 gets none: its proj tiles would race the
                        # previous chunk's last transposes, so its share is
                        # redistributed over heads 1..7 (race-safe at any ti)
                        pe_list = ([] if c == 0 or h == 0 else
                                   list(range((h - 1) * NE // (HQL - 1),
                                              h * NE // (HQL - 1))))
                        D = 3
                        # diag tiles spread through the loop (one after
                        # every c full tiles): their ACT exp work and DVE
                        # tri-adds never bunch at the tail, and the loop
                        # ends on the smallest (128-col) exp.
                        t_seq = []
                        for i in range(4):
                            t_seq += list(range(c * i, c * (i + 1)))
                            t_seq.append(4 * c + i)
                        # pair p becomes ready at the position of its later
                        # tile; emit its AV matmuls D positions later
                        ready = {}
                        for ti, t in enumerate(t_seq):
                            ready[t // 2] = ti
                        pair_seq = sorted(ready, key=lambda p: ready[p])
                        pq = list(pair_seq)
                        npe0 = len(pe_list)
                        for ti, t in enumerate(t_seq):
                            j = t - 4 * c  # >= 0 on diagonal-group tiles
                            col0 = 128 * j if j > 0 else 0
                            ps = psS.tile([128, 512], f32, tag="s")
                            nc.tensor.matmul(
                                ps[:, col0:512],
                                lhsT=kT[:, v, 128 * t:128 * (t + 1)],
                                rhs=qT[:, h, 512 * c + col0:512 * (c + 1)],
                                start=True, stop=True)
                            while carry and carry[0][0] <= ti:
                                carry.pop(0)[1]()
                            if j >= 0:
                                nc.vector.tensor_tensor(
                                    ps[:, 128 * j:128 * (j + 1)],
                                    ps[:, 128 * j:128 * (j + 1)],
                                    tri_sb[:], add)
                            if c > 0:
                                nc.scalar.activation(
                                    pT2[:, t, col0:512], ps[:, col0:512],
                                    Exp, scale=scale, bias=pb_sb[:])
                                while pq and ready[pq[0]] <= ti - D:
                                    vmms_pair(pq.pop(0))
                            else:
                                pT = ppool.tile([128, 512], b16, tag="pT")
                                nc.scalar.activation(
                                    pT[:, col0:512], ps[:, col0:512], Exp,
                                    scale=scale)
                                pTs[t] = pT
                                if ti >= D:
                                    vmms(t_seq[ti - D])
                            if pe_list and \
                               (ti + 1) * npe0 // (n_tk + 2) > \
                               ti * npe0 // (n_tk + 2):
                                proj_tile(pe_list.pop(0), c - 1)
                        for e in pe_list:
                            proj_tile(e, c - 1)

                        # Package this head's tail: the last AV matmuls
                        # (their exps are still draining on ACT) plus the
                        # normalize chain; and, later, the A^T transposes.
                        # Both run inside the NEXT head's t-loop.
                        holder = []

                        def make_tail(vmms=vmms, vmms_pair=vmms_pair,
                                      accs=accs, n_tk=n_tk, D=D, c=c,
                                      holder=holder, t_seq=tuple(t_seq),
                                      pq=pq):
                            def emit():
                                if c > 0:
                                    for p in pq:
                                        vmms_pair(p)
                                else:
                                    for t in t_seq[max(0, n_tk - D):]:
                                        vmms(t)
                                # batched normalize: 4 s-blocks land in one
                                # [128,512] bf16 tile; the fp8 hi/lo split
                                # happens AFTER the transpose (identical
                                # math, and bf16 PE transposes are legal
                                # where fp8 ones need stride-2 outputs)
                                an = npool.tile([128, 512], b16, tag="an")
                                for s in range(4):
                                    rec = spool.tile([128, 1], f32,
                                                     tag="rec")
                                    nc.vector.reciprocal(
                                        rec[:], accs[s][:, 128:129])
                                    nc.vector.tensor_scalar_mul(
                                        an[:, 128 * s:128 * (s + 1)],
                                        accs[s][:, 0:128], rec[:])
                                holder.append(an)
                            return emit

                        def make_tr(holder=holder, h=h, c=c):
                            def emit():
                                pt = psP.tile([128, 512], b16, tag="p8",
                                              name="pt8")
                                an = holder[0]
                                for s in range(4):
                                    nc.tensor.transpose(
                                        pt[:, 128 * s:128 * (s + 1)],
                                        an[:, 128 * s:128 * (s + 1)],
                                        id_sb[:])
                                cs = slice(512 * c, 512 * (c + 1))
                                # post-transpose hi/lo split: hi on ACT,
                                # lo = pt - hi on DVE
                                nc.vector.tensor_copy(aTh[:, h, cs],
                                                      pt[:])
                                nc.vector.tensor_tensor(
                                    aTl[:, h, cs], pt[:], aTh[:, h, cs],
                                    sub)
                            return emit

                        for _, fn in carry:  # flush any unemitted leftovers
                            fn()
                        # emit points must fit inside the NEXT iteration's
                        # t-loop (n_tk=4 when it is a c==0 head)
                        nxt_c0 = (c == 0 and h < HQL - 1)
                        carry = [(1, make_tail()),
                                 (3 if nxt_c0 else 6, make_tr())]

                for _, fn in carry:
                    fn()
                # drain: projection of the last chunk through the 3-bank ring
                for e in range(NE):
                    proj_tile(e, NCH - 1)

    nc.compile()
    return nc


def _rope_tables(T=T):
    j = np.arange(64, dtype=np.float64)
    inv_freq = 1.0 / (BASE_FREQ ** (2.0 * j / HD))
    t = np.arange(T, dtype=np.float64)
    fr = t[:, None] * inv_freq[None, :]          # [T, 64]
    cos = np.cos(fr) / SW                        # fold 1/SW (fp8 w scaling)
    sin = np.sin(fr) / SW
    cos_tbl = np.concatenate([cos, cos], axis=1).T    # [128, T]
    sin_tbl = sin.T                                   # [64, T]
    return cos_tbl.astype(bf16), sin_tbl.astype(bf16)


def _hilo(a):
    """fp8 e4m3 hi/lo split of a float32 array."""
    h = a.astype(f8e4)
    l = (a - h.astype(np.float32)).astype(f8e4)
    return h, l


def _pack_w(w):
    """[KE, M] -> [128, M//128, KE//128, 128]: w_l[p, m, a, j] = w[128a+p, 128m+j]."""
    KE, M = w.shape
    return np.ascontiguousarray(
        w.reshape(KE // 128, 128, M // 128, 128).transpose(1, 2, 0, 3))


def prep_core_inputs(x, wq, wk, wv, wproj):
    cos_tbl, rsin_tbl = _rope_tables()
    tri = np.where(np.arange(128)[None, :] >= np.arange(128)[:, None],
                   0.0, NEG).astype(np.float32)
    ident = np.eye(128, dtype=bf16)

    # shared fp8 splits (sliced per core below)
    wqh, wql = _hilo(wq * SW)
    wkh, wkl = _hilo(wk * SW)
    wvh, wvl = _hilo(wv * SW)
    wph, wpl = _hilo(wproj * SW)

    xs = []
    for b in range(B):
        xt = np.ascontiguousarray(
            x[b].T.reshape(N_EMBD // 128, 128, T).transpose(1, 0, 2))
        xs.append(_hilo(xt.astype(np.float32)))

    def packp(w, cols):
        # [1024, E] -> [128, NE, HQL, 128]: per e-column-tile slabs
        return np.ascontiguousarray(
            w[cols, :].reshape(HQL, 128, N_EMBD // 128, 128)
            .transpose(1, 2, 0, 3))

    in_maps = []
    for ci in range(N_CORES):
        b, g = divmod(ci, TPG)
        qcols = slice(g * HQL * HD, (g + 1) * HQL * HD)
        kvcols = slice(g * HKVL * HD, (g + 1) * HKVL * HD)
        in_maps.append({
            "xh": xs[b][0], "xl": xs[b][1],
            "wqh": _pack_w(wqh[:, qcols]), "wql": _pack_w(wql[:, qcols]),
            "wkh": _pack_w(wkh[:, kvcols]), "wkl": _pack_w(wkl[:, kvcols]),
            "wvh": _pack_w(wvh[:, kvcols]), "wvl": _pack_w(wvl[:, kvcols]),
            "wph": packp(wph, qcols), "wpl": packp(wpl, qcols),
            "cos": cos_tbl, "rsin": rsin_tbl, "tri": tri, "ident": ident,
        })
    return in_maps


_NC_CACHE = {}


def _get_nc():
    if "nc" not in _NC_CACHE:
        _NC_CACHE["nc"] = build_nc()
    return _NC_CACHE["nc"]


def _get_runner():
    """Cached sharded-jit executor over the 8 cores (no donation, so the
    compiled executable is reusable across calls)."""
    if "runner" in _NC_CACHE:
        return _NC_CACHE["runner"]
    import jax
    from jax.sharding import Mesh, PartitionSpec, NamedSharding
    from jax.experimental.shard_map import shard_map
    from concourse import mybir
    from concourse.bass2jax import (_bass_exec_p, install_neuronx_cc_hook,
                                    partition_id_tensor)

    nc = _get_nc()
    install_neuronx_cc_hook()
    pname = nc.partition_id_tensor.name if nc.partition_id_tensor else None
    in_names, out_names, out_avals, zero_shapes = [], [], [], []
    for alloc in nc.m.functions[0].allocations:
        if not isinstance(alloc, mybir.MemoryLocationSet):
            continue
        name = alloc.memorylocations[0].name
        if alloc.kind == "ExternalInput":
            if name != pname:
                in_names.append(name)
        elif alloc.kind == "ExternalOutput":
            out_names.append(name)
            shape = tuple(alloc.tensor_shape)
            dtype = mybir.dt.np(alloc.dtype)
            out_avals.append(jax.core.ShapedArray(shape, dtype))
            zero_shapes.append((shape, dtype))
    all_names = in_names + out_names + ([pname] if pname else [])

    def _body(*args):
        operands = list(args)
        if pname:
            operands.append(partition_id_tensor())
        return tuple(_bass_exec_p.bind(
            *operands, out_avals=tuple(out_avals), in_names=tuple(all_names),
            out_names=tuple(out_names), lowering_input_output_aliases=(),
            sim_require_finite=True, sim_require_nnan=True, nc=nc))

    devices = jax.devices()[:N_CORES]
    mesh = Mesh(np.asarray(devices), ("core",))
    nin = len(in_names) + len(out_names)
    sharded = jax.jit(
        shard_map(_body, mesh=mesh, in_specs=(PartitionSpec("core"),) * nin,
                  out_specs=(PartitionSpec("core"),) * len(out_names),
                  check_rep=False),
        keep_unused=True)
    sh = NamedSharding(mesh, PartitionSpec("core"))
    zeros = [jax.device_put(
        np.zeros((N_CORES * s[0], *s[1:]), dt), sh)
        for s, dt in zero_shapes]

    def run(in_maps):
        concat = [np.concatenate([m[n] for m in in_maps], axis=0)
                  for n in in_names]
        dev_in = [jax.device_put(a, sh) for a in concat]
        outs = sharded(*dev_in, *zeros)
        jax.block_until_ready(outs)
        return [
            {n: np.asarray(outs[i]).reshape(N_CORES, *out_avals[i].shape)[ci]
             for i, n in enumerate(out_names)}
            for ci in range(N_CORES)]

    _NC_CACHE["runner"] = run
    return run


def kernel(x, wq, wk, wv, wproj):
    in_maps = prep_core_inputs(np.asarray(x, dtype=np.float32),
                               np.asarray(wq, dtype=np.float32),
                               np.asarray(wk, dtype=np.float32),
                               np.asarray(wv, dtype=np.float32),
                               np.asarray(wproj, dtype=np.float32))
    results = _get_runner()(in_maps)
    y = np.empty((B, T, N_EMBD), dtype=np.float32)
    for b in range(B):
        acc = results[b * TPG]["yt"].copy()
        for g in range(1, TPG):
            acc += results[b * TPG + g]["yt"]
        y[b] = acc.T / SW
    return y


if __name__ == "__main__":
    rng = np.random.default_rng(0)
    x = rng.standard_normal((B, T, N_EMBD), dtype=np.float32)
    wq_ = (rng.standard_normal((N_EMBD, N_EMBD), dtype=np.float32) * 0.02)
    wk_ = (rng.standard_normal((N_EMBD, HKV * HD), dtype=np.float32) * 0.02)
    wv_ = (rng.standard_normal((N_EMBD, HKV * HD), dtype=np.float32) * 0.02)
    wp_ = (rng.standard_normal((N_EMBD, N_EMBD), dtype=np.float32) * 0.02)
    y = kernel(x, wq_, wk_, wv_, wp_)
    print("out", y.shape, y.dtype, float(np.abs(y).max()))
